# revision 1
# baseline (speedup 1.0000x reference)
"""Trainium2 Bass kernel for nn_Encoder_88691074663154 (dense transformer encoder layer).

Strategy v2: batch x sequence sharding, ZERO collectives. Core c = (b, s) with
b = c // 2 owning batch b and s = c % 2 owning half its sequence (1024 query
tokens). Attention keys are per-batch, so each core recomputes K and V for its
batch's full 2048 positions locally (~27us of PE time) instead of AllGathering
them (~530us of collective time in the baseline). The host rotates each core's
token order so its own 1024 query tokens are always columns 0:1023 -> one
uniform SPMD program; softmax over keys is order-invariant so the rotation
does not change results.

Layout: activations feature-major (features on partitions, tokens free) so all
matmuls are transpose-free. V is token-major with a ones column interleaved
per head (65-wide) so the softmax denominator falls out of the AV matmul (row
64). All matmul operands are bf16 (same PE rate as f32r, half the SBUF/DMA);
accumulation stays fp32 in PSUM; residuals/LN statistics stay fp32 (f32r
bitcast for the ones-column stat matmuls).
"""
import os
import sys

sys.path.insert(0, "/opt/trn_rl_repo")

import numpy as np

import concourse.bacc as bacc
import concourse.mybir as mybir
import concourse.tile as tile
from concourse.bass_utils import run_bass_kernel_spmd

F32 = mybir.dt.float32
F32R = mybir.dt.float32r
BF16 = mybir.dt.bfloat16
AF = mybir.ActivationFunctionType
OP = mybir.AluOpType

L, B, E, H, HD, HID = 2048, 4, 512, 8, 64, 2048
NCORES = 8
TOK = 1024                # own query tokens per core
EC = E // 128             # 4 feature chunks
HIDC = HID // 128         # 16 hidden chunks
KT = L // 128             # 16 key chunks

_BUILD_CACHE = {}


def build_encoder():
    if "nc" in _BUILD_CACHE:
        return _BUILD_CACHE["nc"]
    nc = bacc.Bacc(None, num_devices=NCORES)

    # ---- DRAM parameters (per core) ----
    xT_in = nc.declare_dram_parameter("xT", [E, L], BF16, isOutput=False)
    xTf_in = nc.declare_dram_parameter("xTf", [E, TOK], F32, isOutput=False)
    pe_in = nc.declare_dram_parameter("pe2d", [128, EC], F32, isOutput=False)
    wqkv_in = nc.declare_dram_parameter("wqkvT", [E, 3 * E], BF16, isOutput=False)
    wo_in = nc.declare_dram_parameter("woT", [E, E], BF16, isOutput=False)
    w1_in = nc.declare_dram_parameter("w1T", [E, HID], BF16, isOutput=False)
    w2_in = nc.declare_dram_parameter("w2T", [HID, E], BF16, isOutput=False)
    bqkv_in = nc.declare_dram_parameter("bqkv2d", [128, 3 * EC], F32, isOutput=False)
    bvr_in = nc.declare_dram_parameter("bv_rep", [128, E], F32, isOutput=False)
    bo_in = nc.declare_dram_parameter("bo2d", [128, EC], F32, isOutput=False)
    b1_in = nc.declare_dram_parameter("b1_2d", [128, HIDC], F32, isOutput=False)
    b2_in = nc.declare_dram_parameter("b2_2d", [128, EC], F32, isOutput=False)
    g_in = nc.declare_dram_parameter("g2d", [128, EC], F32, isOutput=False)
    bb_in = nc.declare_dram_parameter("bb2d", [128, EC], F32, isOutput=False)
    ones_in = nc.declare_dram_parameter("ones_row", [1, 128], F32, isOutput=False)
    onesc_in = nc.declare_dram_parameter("ones_col", [128, 1], F32, isOutput=False)
    yT_out = nc.declare_dram_parameter("yT", [E, TOK], F32, isOutput=True)

    with tile.TileContext(nc) as tc:
        from contextlib import ExitStack
        with ExitStack() as ctx:
            pers = ctx.enter_context(tc.tile_pool(name="pers", bufs=1))

            # ---- persistent tiles ----
            onr = pers.tile([1, 128], F32R, tag="onr")
            onc = pers.tile([128, 1], F32R, tag="onc")
            pe2 = pers.tile([128, EC], F32, tag="pe2")
            bqkv = pers.tile([128, 3 * EC], F32, tag="bqkv")
            bvr = pers.tile([128, E], F32, tag="bvr")
            bo2d = pers.tile([128, EC], F32, tag="bo2d")
            b12d = pers.tile([128, HIDC], F32, tag="b12d")
            b22d = pers.tile([128, EC], F32, tag="b22d")
            g2d = pers.tile([128, EC], F32, tag="g2d")
            bb2d = pers.tile([128, EC], F32, tag="bb2d")

            xw = [pers.tile([128, TOK], F32, tag=f"xw{k}", name=f"xw{k}") for k in range(EC)]
            kTt = [pers.tile([128, L], BF16, tag=f"kT{p}", name=f"kT{p}") for p in range(EC)]
            qTt = [pers.tile([128, TOK], BF16, tag=f"qT{p}", name=f"qT{p}") for p in range(EC)]
            vt = [pers.tile([128, H * 65], BF16, tag=f"vt{t}", name=f"vt{t}") for t in range(KT)]
            woT = [pers.tile([128, E], BF16, tag=f"woT{k}", name=f"woT{k}") for k in range(EC)]
            w1T = [pers.tile([128, HID], BF16, tag=f"w1T{k}", name=f"w1T{k}") for k in range(EC)]
            w2T = [pers.tile([128, E], BF16, tag=f"w2T{k}", name=f"w2T{k}") for k in range(HIDC)]
            oT = [pers.tile([128, TOK], BF16, tag=f"oT{p}", name=f"oT{p}") for p in range(EC)]

            # ========== Stage Q+A: QKV interleaved with attention ==========
            # All Q/K/V matmul outputs share the scores PSUM ring (tag "sc"),
            # so PSUM stays at 8 banks: sc ring 2x2 + pso 2x2. K/Q of head
            # pair p+1 are emitted between heads so the PE stays fed during
            # the ACT-bound (exp) stretches; V is computed inside head 0's
            # key loop, just ahead of each AV step.
            nc.sync.dma_start(pe2[:], pe_in[:])
            with tc.tile_pool(name="pq", bufs=1) as pq, \
                 tc.tile_pool(name="pa", bufs=1) as pa, \
                 tc.tile_pool(name="ps_sc", bufs=2, space="PSUM") as ps_sc, \
                 tc.tile_pool(name="ps_o", bufs=2, space="PSUM") as ps_o:
                xr = [pq.tile([128, L], BF16, tag=f"xr{k}", name=f"xr{k}") for k in range(EC)]
                xrf = [pq.tile([128, TOK], F32, tag=f"xrf{k}", name=f"xrf{k}") for k in range(EC)]
                wqkvT = [pq.tile([128, 3 * E], BF16, tag=f"wqkv{k}", name=f"wqkv{k}") for k in range(EC)]
                xa = [pq.tile([128, L], BF16, tag=f"xa{k}", name=f"xa{k}") for k in range(EC)]

                for k in range(EC):
                    nc.sync.dma_start(xr[k][:], xT_in[k * 128:(k + 1) * 128, :])
                    nc.sync.dma_start(wqkvT[k][:], wqkv_in[k * 128:(k + 1) * 128, :])
                nc.sync.dma_start(bqkv[:], bqkv_in[:])
                nc.sync.dma_start(bvr[:], bvr_in[:])
                for k in range(EC):
                    # x + pe (pe is per-feature here: one batch per core)
                    nc.vector.tensor_scalar_add(xa[k][:], xr[k][:], pe2[:, k:k + 1])
                # fp32 residual of own tokens: DMA now, add after head 0 so the
                # DVE queue isn't blocked ahead of the K/Q bias copies
                for k in range(EC):
                    nc.sync.dma_start(xrf[k][:], xTf_in[k * 128:(k + 1) * 128, :])

                def kq_group(p, g, is_k):
                    # one 512-column group of K (g<4) or Q (g<2) for head pair p
                    sl = slice(g * 512, (g + 1) * 512)
                    col = E + p * 128 if is_k else p * 128
                    dst = kTt[p] if is_k else qTt[p]
                    bcol = EC + p if is_k else p
                    ps = ps_sc.tile([128, TOK], F32, tag="sc", name="ps_kq")
                    for k in range(EC):
                        nc.tensor.matmul(
                            ps[:, 0:512], wqkvT[k][:, col:col + 128],
                            xa[k][:, sl], start=(k == 0), stop=(k == EC - 1))
                    nc.vector.tensor_scalar_add(dst[:, sl], ps[:, 0:512],
                                                bqkv[:, bcol:bcol + 1])

                def kq_chunk(p):
                    for g in range(4):
                        kq_group(p, g, True)
                    for g in range(2):
                        kq_group(p, g, False)

                def v_chunk(t):
                    ps = ps_sc.tile([128, TOK], F32, tag="sc", name="ps_v")
                    for k in range(EC):
                        nc.tensor.matmul(
                            ps[:, 0:512], xa[k][:, t * 128:(t + 1) * 128],
                            wqkvT[k][:, 2 * E:3 * E], start=(k == 0), stop=(k == EC - 1))
                    vr = vt[t][:].rearrange("t (h c) -> t h c", c=65)
                    nc.vector.tensor_add(
                        vr[:, :, 0:64],
                        ps[:, 0:512].rearrange("t (h c) -> t h c", c=64),
                        bvr[:].rearrange("t (h c) -> t h c", c=64))
                    nc.vector.memset(vr[:, :, 64:65], 1.0)

                def normalize(h, pso):
                    # 1/rowsum (psum row 64); replicate across partitions on the
                    # idle gpsimd engine (no PSUM, no PE -> never stalls the
                    # in-order PE stream or the scores PSUM ring).
                    hp, ro = h // 2, (h % 2) * 64
                    rr = pa.tile([1, TOK], F32R, tag="rr", bufs=2, name="rr")
                    with nc.allow_low_precision(reason="softmax denom rounded to f32r"):
                        nc.vector.reciprocal(rr[:], pso[64:65, :])
                    rsb = pa.tile([64, TOK], F32R, tag="rsb", bufs=2, name="rsb")
                    nc.gpsimd.partition_broadcast(rsb[:], rr[:])
                    nc.vector.tensor_mul(oT[hp][ro:ro + 64, :], pso[0:64, :],
                                         rsb[:].bitcast(F32))

                for p in range(EC):
                    kq_chunk(p)

                def s_exp_h(h, t, v_inline=False):
                    # scores for key chunk t + its exp; returns the ex tile
                    hp, ro = h // 2, (h % 2) * 64
                    if v_inline:
                        v_chunk(t)
                    pssc = ps_sc.tile([128, TOK], F32, tag="sc", name="ps_sc")
                    for g in range(2):
                        sl = slice(g * 512, (g + 1) * 512)
                        nc.tensor.matmul(
                            pssc[:, sl],
                            kTt[hp][ro:ro + 64, t * 128:(t + 1) * 128],
                            qTt[hp][ro:ro + 64, sl],
                            start=True, stop=True)
                    ex = pa.tile([128, TOK], BF16, tag="ex", bufs=4, name="ex")
                    nc.scalar.activation(ex[:], pssc[:], AF.Exp, scale=0.125)
                    return ex

                def av(h, t, pso, ex):
                    for g in range(2):
                        sl = slice(g * 512, (g + 1) * 512)
                        nc.tensor.matmul(
                            pso[:, sl], vt[t][:, h * 65:(h + 1) * 65], ex[:, sl],
                            start=(t == 0), stop=(t == KT - 1))

                # heads 0+1 run one interleaved key loop: alone, head 0's
                # inline V-compute makes it PE-bound with ACT half idle; the
                # pair feeds ACT two exps per key chunk. Their AV accumulators
                # are exactly the two ps_o ring slots.
                pso0 = ps_o.tile([65, TOK], F32, tag="pso", name="ps_av")
                pso1 = ps_o.tile([65, TOK], F32, tag="pso", name="ps_av")
                ex0 = s_exp_h(0, 0, v_inline=True)
                ex1 = s_exp_h(1, 0)
                for t in range(KT):
                    nx0 = s_exp_h(0, t + 1, v_inline=True) if t + 1 < KT else None
                    nx1 = s_exp_h(1, t + 1) if t + 1 < KT else None
                    av(0, t, pso0, ex0)
                    av(1, t, pso1, ex1)
                    ex0, ex1 = nx0, nx1
                normalize(0, pso0)
                normalize(1, pso1)
                # post-phase weights (DMA is idle during attention)
                for k in range(EC):
                    nc.sync.dma_start(woT[k][:], wo_in[k * 128:(k + 1) * 128, :])
                    nc.sync.dma_start(w1T[k][:], w1_in[k * 128:(k + 1) * 128, :])
                for k in range(HIDC):
                    nc.sync.dma_start(w2T[k][:], w2_in[k * 128:(k + 1) * 128, :])
                nc.sync.dma_start(bo2d[:], bo_in[:])
                nc.sync.dma_start(b12d[:], b1_in[:])
                nc.sync.dma_start(b22d[:], b2_in[:])
                nc.sync.dma_start(g2d[:], g_in[:])
                nc.sync.dma_start(bb2d[:], bb_in[:])
                nc.sync.dma_start(onr[:], ones_in[:].bitcast(F32R))
                nc.sync.dma_start(onc[:], onesc_in[:].bitcast(F32R))

                for h in range(2, H):
                    pso = ps_o.tile([65, TOK], F32, tag="pso", name="ps_av")
                    # software-pipelined: scores(t+1) are emitted BEFORE av(t),
                    # so the in-order PE never idles behind an AV that waits on
                    # exp(t) -- it computes the next chunk's scores instead.
                    ex_t = s_exp_h(h, 0)
                    for t in range(KT):
                        ex_n = s_exp_h(h, t + 1) if t + 1 < KT else None
                        av(h, t, pso, ex_t)
                        ex_t = ex_n
                    normalize(h, pso)
                    if h == 6:
                        # x+pe fp32 residual, late: keeps early DVE free
                        for k in range(EC):
                            nc.vector.tensor_scalar_add(xw[k][:], xrf[k][:], pe2[:, k:k + 1])

            # ================= Stage P: out-proj + LN1 + FFN + LN2 =================
            # Half-pipelined: LN chains (DVE/Pool/ACT) for one column half run
            # while the PE works on the other half's matmuls.
            with tc.tile_pool(name="pp", bufs=1) as pp, \
                 tc.tile_pool(name="ps_mm", bufs=2, space="PSUM") as ps_mm, \
                 tc.tile_pool(name="ps_ln", bufs=1, space="PSUM") as ps_ln:

                xres = [pp.tile([128, TOK], F32R, tag=f"xres{m}", name=f"xres{m}") for m in range(EC)]
                x1b = [pp.tile([128, TOK], BF16, tag=f"x1b{m}", name=f"x1b{m}") for m in range(EC)]
                yt = [pp.tile([128, TOK], F32, tag=f"yt{m}", name=f"yt{m}") for m in range(EC)]
                hT = [pp.tile([128, TOK], BF16, tag=f"hT{m}", name=f"hT{m}") for m in range(HIDC)]

                _ln = {}

                def ln_stats(src, gh):
                    """Stats + per-token scalar chain for one column half."""
                    sl = slice(gh * 512, (gh + 1) * 512)
                    sqs = []
                    for k in range(EC):
                        sq = pp.tile([128, 512], F32R, tag="sq", bufs=2, name="sq")
                        nc.scalar.activation(sq[:], src[k][:, sl].bitcast(F32), AF.Square)
                        sqs.append(sq)
                    pss = ps_ln.tile([1, 512], F32, tag="pss", name="ps_mean")
                    for k in range(EC):
                        nc.tensor.matmul(pss[:], onc[:], src[k][:, sl],
                                         start=(k == 0), stop=(k == EC - 1))
                    pss2 = ps_ln.tile([1, 512], F32, tag="pss2", name="ps_var")
                    for k in range(EC):
                        nc.tensor.matmul(pss2[:], onc[:], sqs[k][:],
                                         start=(k == 0), stop=(k == EC - 1))
                    # ones_col carries 1/E, so pss/pss2 are already E[x], E[x^2]
                    rows = pp.tile([1, 2 * 512], F32, tag="lnrows", bufs=2, name="lnrows")
                    rowsr = pp.tile([1, 2 * 512], F32R, tag="lnrowsr", bufs=2, name="lnrowsr")
                    mu2 = rows[0:1, 0:512]
                    rec = rows[0:1, 512:1024]
                    mur = rowsr[0:1, 0:512]
                    rsq = rowsr[0:1, 512:1024]
                    nc.vector.tensor_copy(mur, pss[:])
                    nc.scalar.activation(mu2, pss[:], AF.Square)
                    nc.vector.scalar_tensor_tensor(rec, pss2[:], 1.0, mu2,
                                                   op0=OP.mult, op1=OP.subtract)
                    nc.vector.tensor_scalar_add(rec, rec, 1e-5)
                    nc.vector.reciprocal(rec, rec)
                    nc.scalar.activation(rsq, rec, AF.Sqrt)
                    _ln[gh] = (mur, rsq)

                def ln_finish(src, dst, gh, dma=False):
                    """Replicate + normalize one column half; optionally stream out.
                    Both replicated rows are copied to SBUF once, then each
                    feature chunk's 3-op chain runs entirely on ONE engine,
                    alternating DVE/gpsimd so the two proceed in parallel."""
                    sl = slice(gh * 512, (gh + 1) * 512)
                    mur, rsq = _ln[gh]
                    psm = ps_ln.tile([128, TOK], F32, tag="psm", name="ps_lnrep")
                    nc.tensor.matmul(psm[:, 0:512], onr[:], mur, start=True, stop=True)
                    nc.tensor.matmul(psm[:, 512:1024], onr[:], rsq, start=True, stop=True)
                    msb = pp.tile([128, 512], F32, tag="lnmsb", bufs=1, name="lnmsb")
                    rsb = pp.tile([128, 512], F32, tag="lnrsb", bufs=1, name="lnrsb")
                    nc.vector.tensor_copy(msb[:], psm[:, 0:512])
                    nc.vector.tensor_copy(rsb[:], psm[:, 512:1024])
                    for k in range(EC):
                        eng = nc.vector if k % 2 == 0 else nc.gpsimd
                        t1 = pp.tile([128, 512], F32, tag="t1", bufs=2, name="t1")
                        eng.tensor_sub(t1[:], src[k][:, sl].bitcast(F32), msb[:])
                        t2 = pp.tile([128, 512], F32, tag="t2", bufs=2, name="t2")
                        eng.tensor_mul(t2[:], t1[:], rsb[:])
                        eng.tensor_scalar(dst[k][:, sl], t2[:], g2d[:, k:k + 1],
                                          bb2d[:, k:k + 1], op0=OP.mult, op1=OP.add)
                        if dma:
                            nc.sync.dma_start(yT_out[k * 128:(k + 1) * 128, sl], dst[k][:, sl])

                def proj(g):
                    sl = slice(g * 512, (g + 1) * 512)
                    for m in range(EC):
                        pst = ps_mm.tile([128, 512], F32, tag="mm", name="ps_proj")
                        for k in range(EC):
                            nc.tensor.matmul(pst[:], woT[k][:, m * 128:(m + 1) * 128],
                                             oT[k][:, sl], start=(k == 0), stop=(k == EC - 1))
                        nc.vector.scalar_tensor_tensor(
                            xres[m][:, sl], pst[:], bo2d[:, m:m + 1], xw[m][:, sl],
                            op0=OP.add, op1=OP.add)

                def ffn1(g):
                    sl = slice(g * 512, (g + 1) * 512)
                    for m in range(HIDC):
                        psf = ps_mm.tile([128, 512], F32, tag="mm", name="ps_f1")
                        for k in range(EC):
                            nc.tensor.matmul(psf[:], w1T[k][:, m * 128:(m + 1) * 128],
                                             x1b[k][:, sl], start=(k == 0), stop=(k == EC - 1))
                        nc.scalar.activation(hT[m][:, sl], psf[:], AF.Gelu,
                                             bias=b12d[:, m:m + 1])

                def ffn2(g):
                    # xf overwrites xres tiles; residual read from bf16 x1b
                    sl = slice(g * 512, (g + 1) * 512)
                    for m in range(EC):
                        psg = ps_mm.tile([128, 512], F32, tag="mm", name="ps_f2")
                        for k2 in range(HIDC):
                            nc.tensor.matmul(psg[:], w2T[k2][:, m * 128:(m + 1) * 128],
                                             hT[k2][:, sl], start=(k2 == 0), stop=(k2 == HIDC - 1))
                        nc.vector.scalar_tensor_tensor(
                            xres[m][:, sl], psg[:], b22d[:, m:m + 1], x1b[m][:, sl],
                            op0=OP.add, op1=OP.add)

                proj(0)
                ln_stats(xres, 0)
                proj(1)
                ln_finish(xres, x1b, 0)
                ln_stats(xres, 1)
                ffn1(0)
                ln_finish(xres, x1b, 1)
                ffn1(1)
                ffn2(0)
                ln_stats(xres, 0)
                ffn2(1)
                ln_finish(xres, yt, 0, dma=True)
                ln_stats(xres, 1)
                ln_finish(xres, yt, 1, dma=True)

    nc.compile()
    _BUILD_CACHE["nc"] = nc
    return nc


def _pos_encoding_np(S, Emb):
    t = np.arange(S, dtype=np.float32)[:, None]
    i = np.arange(Emb, dtype=np.float32)[None, :]
    even = np.sin((t + 1.0) * np.power(np.float32(10000.0), -i / Emb))
    odd = np.cos((t + 1.0) * np.power(np.float32(10000.0), -(i + 1.0) / Emb))
    return np.where(np.arange(Emb)[None, :] % 2 == 0, even, odd).astype(np.float32)


def prepare_in_maps(x, in_proj_w, in_proj_b, out_w, out_b, w1, b1, w2, b2, ln_g, ln_b):
    import ml_dtypes
    bf16 = ml_dtypes.bfloat16
    pe = _pos_encoding_np(B, E)                      # (B, E)
    wq, wk, wv = in_proj_w[:E], in_proj_w[E:2 * E], in_proj_w[2 * E:]
    shared = {
        "wqkvT": np.ascontiguousarray(
            np.concatenate([wq.T, wk.T, wv.T], axis=1)).astype(bf16),
        "woT": np.ascontiguousarray(out_w.T).astype(bf16),
        "w1T": np.ascontiguousarray(w1.T).astype(bf16),
        "w2T": np.ascontiguousarray(w2.T).astype(bf16),
        "bqkv2d": np.ascontiguousarray(in_proj_b.reshape(3 * EC, 128).T),
        "bv_rep": np.ascontiguousarray(np.tile(in_proj_b[2 * E:], (128, 1))),
        "bo2d": np.ascontiguousarray(out_b.reshape(EC, 128).T),
        "b1_2d": np.ascontiguousarray(b1.reshape(HIDC, 128).T),
        "b2_2d": np.ascontiguousarray(b2.reshape(EC, 128).T),
        "g2d": np.ascontiguousarray(ln_g.reshape(EC, 128).T),
        "bb2d": np.ascontiguousarray(ln_b.reshape(EC, 128).T),
        "ones_row": np.ones((1, 128), np.float32),
        "ones_col": np.full((128, 1), 1.0 / E, np.float32),  # LN stats: mean in one matmul
    }
    in_maps = []
    for c in range(NCORES):
        b, s = c // 2, c % 2
        xb = x[:, b, :]                              # (L, E)
        # rotate so own tokens are first: [own 1024 | other 1024]
        xrot = np.concatenate([xb[s * TOK:(s + 1) * TOK], xb[(1 - s) * TOK:(2 - s) * TOK]], axis=0)
        m = dict(shared)
        m["xT"] = np.ascontiguousarray(xrot.T).astype(bf16)   # (E, L)
        m["xTf"] = np.ascontiguousarray(xb[s * TOK:(s + 1) * TOK].T)  # (E, TOK) fp32
        m["pe2d"] = np.ascontiguousarray(pe[b].reshape(EC, 128).T)
        in_maps.append(m)
    return in_maps


def assemble_output(results):
    y = np.empty((L, B, E), np.float32)
    for c in range(NCORES):
        b, s = c // 2, c % 2
        y[s * TOK:(s + 1) * TOK, b, :] = results[c]["yT"].T
    return y


def kernel(**inputs):
    inputs = {k: np.asarray(v, dtype=np.float32) for k, v in inputs.items()}
    nc = build_encoder()
    in_maps = prepare_in_maps(**inputs)
    res = run_bass_kernel_spmd(nc, in_maps, core_ids=list(range(NCORES)))
    return assemble_output(res.results)



# revision 10
# speedup vs baseline: 1.1471x; 1.1471x over previous
"""Trainium2 Bass kernel for nn_Encoder_88691074663154 (dense transformer encoder layer).

Strategy v3: batch x sequence sharding (core c = (b, s): batch c//2, sequence
half c%2), ZERO collectives; K/V recomputed per core for the full 2048 keys.
The host rotates each core's token order so its own 1024 query tokens are
always columns 0:1023 -> one uniform SPMD program.

Perf layout on top of v2:
- QKV / AV / FFN1 / FFN2 matmuls run in fp8e4 with MatmulPerfMode.DoubleRow:
  operands are [128, 2, N] pairs of 128-deep contraction tiles, costing
  0.5 cycles per output column -> 4x bf16 throughput. Weights are scaled by
  64 on the host (fp8e4 subnormal floor) and the 1/64 is folded into the
  existing psum-readout ops (tensor_scalar / STT / gelu's activation scale).
  Scores and out-proj stay bf16; LN stats stay f32r.
- softmax exp is split across three engines: ACT (table exp, fp8 out) plus
  DVE and GPSIMD computing a Schraudolph exp: u8 = round(log2(e)*s + b)
  IS the fp8e4 bit pattern of exp(s/8) (one affine op per tile). The ~3%
  approximation error cancels through softmax normalization (verified
  end-to-end: no measurable accuracy change).
- b2 (FFN2 bias) is folded into the LN1 bias of the bf16 residual copy, so
  the FFN2 readout stays one STT op. A separate fp8 copy of LN1 output (x18)
  feeds FFN1.
"""
import os
import sys

sys.path.insert(0, "/opt/trn_rl_repo")

import numpy as np

import concourse.bacc as bacc
import concourse.mybir as mybir
import concourse.tile as tile
from concourse.bass_utils import run_bass_kernel_spmd

F32 = mybir.dt.float32
F32R = mybir.dt.float32r
BF16 = mybir.dt.bfloat16
FP8 = mybir.dt.float8e4
U8 = mybir.dt.uint8
AF = mybir.ActivationFunctionType
OP = mybir.AluOpType
PM = mybir.MatmulPerfMode

L, B, E, H, HD, HID = 2048, 4, 512, 8, 64, 2048
NCORES = 8
TOK = 1024                # own query tokens per core
EC = E // 128             # 4 feature chunks
ECP = EC // 2             # 2 feature chunk pairs
HIDC = HID // 128         # 16 hidden chunks
HIDP = HIDC // 2          # 8 hidden chunk pairs
KT = L // 128             # 16 key chunks
KP = KT // 2              # 8 key chunk pairs

WS = 64.0                 # fp8 weight scale
RWS = 1.0 / WS
# Schraudolph exp -> fp8e4 bits: u8 = round(EXP_A * s + EXP_B) where s is the
# raw qk psum (the /8 softmax scale is folded into EXP_A).
EXP_A = 12102203.161561485 / 2**20 / 8.0
EXP_B = 55.650580406188965 - 0.12

# exp engine schedule per head: A=ACT table exp, D=DVE schraudolph
# (gpsimd cannot read PSUM, so it gets the SBUF-side work instead)
EXPAT01 = "ADADADAADADAADAA"   # heads 0/1: A10 D6 (DVE busy with startup)
EXPAT = "ADADADADADADADAA"     # heads 2-7: A9 D7

_BUILD_CACHE = {}


def build_encoder():
    if "nc" in _BUILD_CACHE:
        return _BUILD_CACHE["nc"]
    nc = bacc.Bacc(None, num_devices=NCORES)

    # ---- DRAM parameters (per core) ----
    xT_in = nc.declare_dram_parameter("xT", [E, L], BF16, isOutput=False)
    xTf_in = nc.declare_dram_parameter("xTf", [E, TOK], F32, isOutput=False)
    pe_in = nc.declare_dram_parameter("pe2d", [128, EC], F32, isOutput=False)
    wqkv8_in = nc.declare_dram_parameter("wqkv8", [ECP * 128, 2 * 3 * E], FP8, isOutput=False)
    wo_in = nc.declare_dram_parameter("woT", [E, E], BF16, isOutput=False)
    w18_in = nc.declare_dram_parameter("w18", [ECP * 128, 2 * HID], FP8, isOutput=False)
    w28_in = nc.declare_dram_parameter("w28", [HIDP * 128, 2 * E], FP8, isOutput=False)
    bqkv_in = nc.declare_dram_parameter("bqkv2d", [128, 3 * EC], F32, isOutput=False)
    bo_in = nc.declare_dram_parameter("bo2d", [128, EC], F32, isOutput=False)
    b1_in = nc.declare_dram_parameter("b1_2d", [128, HIDC], F32, isOutput=False)
    g_in = nc.declare_dram_parameter("g2d", [128, EC], F32, isOutput=False)
    bb_in = nc.declare_dram_parameter("bb2d", [128, EC], F32, isOutput=False)
    bbb2_in = nc.declare_dram_parameter("bb2d_b2", [128, EC], F32, isOutput=False)
    ones_in = nc.declare_dram_parameter("ones_row", [1, 128], F32, isOutput=False)
    onesc_in = nc.declare_dram_parameter("ones_col", [128, 1], F32, isOutput=False)
    yT_out = nc.declare_dram_parameter("yT", [E, TOK], F32, isOutput=True)

    with tile.TileContext(nc) as tc:
        from contextlib import ExitStack
        with ExitStack() as ctx:
            pers = ctx.enter_context(tc.tile_pool(name="pers", bufs=1))

            # ---- persistent tiles ----
            onr = pers.tile([1, 128], F32R, tag="onr")
            onc = pers.tile([128, 1], F32R, tag="onc")
            pe2 = pers.tile([128, EC], F32, tag="pe2")
            bqkv = pers.tile([128, 3 * EC], F32, tag="bqkv")
            bo2d = pers.tile([128, EC], F32, tag="bo2d")
            b12d = pers.tile([128, HIDC], F32, tag="b12d")
            g2d = pers.tile([128, EC], F32, tag="g2d")
            bb2d = pers.tile([128, EC], F32, tag="bb2d")
            bbb2 = pers.tile([128, EC], F32, tag="bbb2")

            xw = [pers.tile([128, TOK], F32, tag=f"xw{k}", name=f"xw{k}") for k in range(EC)]
            kTt = [pers.tile([128, L], BF16, tag=f"kT{p}", name=f"kT{p}") for p in range(EC)]
            qTt = [pers.tile([128, TOK], BF16, tag=f"qT{p}", name=f"qT{p}") for p in range(EC)]
            # V pairs: [128, 2, H*66] fp8: per head 64 values + ones col (for
            # the softmax denominator) + 1 pad col so dual-fp8 Ldweights strides
            # are even / 16B-aligned
            vt8 = [pers.tile([128, 2 * H * 66], FP8, tag=f"vt{t}", name=f"vt{t}") for t in range(KP)]
            woT = [pers.tile([128, E], BF16, tag=f"woT{k}", name=f"woT{k}") for k in range(EC)]
            w18p = [pers.tile([128, 2 * HID], FP8, tag=f"w18{k}", name=f"w18{k}") for k in range(ECP)]
            w28p = [pers.tile([128, 2 * E], FP8, tag=f"w28{k}", name=f"w28{k}") for k in range(HIDP)]
            oT = [pers.tile([128, TOK], BF16, tag=f"oT{p}", name=f"oT{p}") for p in range(EC)]

            def pair(t):
                return t[:].rearrange("p (i x) -> p i x", i=2)

            # ========== Stage Q+A: QKV interleaved with attention ==========
            nc.sync.dma_start(pe2[:], pe_in[:])
            with tc.tile_pool(name="pq", bufs=1) as pq, \
                 tc.tile_pool(name="pa", bufs=1) as pa, \
                 tc.tile_pool(name="ps_sc", bufs=2, space="PSUM") as ps_sc, \
                 tc.tile_pool(name="ps_o", bufs=2, space="PSUM") as ps_o:
                xr = [pq.tile([128, L], BF16, tag=f"xr{k}", name=f"xr{k}") for k in range(EC)]
                xrf = [pq.tile([128, TOK], F32, tag=f"xrf{k}", name=f"xrf{k}") for k in range(EC)]
                wq8p = [pq.tile([128, 2 * 3 * E], FP8, tag=f"wq8{k}", name=f"wq8{k}") for k in range(ECP)]
                xap = [pq.tile([128, 2 * L], FP8, tag=f"xa{k}", name=f"xa{k}") for k in range(ECP)]

                for k in range(EC):
                    nc.sync.dma_start(xr[k][:], xT_in[k * 128:(k + 1) * 128, :])
                for kp in range(ECP):
                    nc.sync.dma_start(wq8p[kp][:], wqkv8_in[kp * 128:(kp + 1) * 128, :])
                nc.sync.dma_start(bqkv[:], bqkv_in[:])
                for k in range(EC):
                    # x + pe -> fp8 pair layout (pe is per-feature: one batch/core)
                    # on gpsimd: SBUF-only op, keeps DVE/ACT free for psum readouts
                    with nc.allow_low_precision(reason="fp8 activations for DR matmul"):
                        nc.gpsimd.tensor_scalar_add(
                            pair(xap[k // 2])[:, k % 2, :], xr[k][:], pe2[:, k:k + 1])
                # ones columns of V (written once; disjoint from the value cols)
                for tp in range(KP):
                    vr = vt8[tp][:].rearrange("p (i h c) -> p i h c", i=2, c=66)
                    nc.vector.memset(vr[:, :, :, 64:65], 1.0)
                # fp32 residual of own tokens
                for k in range(EC):
                    nc.sync.dma_start(xrf[k][:], xTf_in[k * 128:(k + 1) * 128, :])

                def kq_group(p, g, is_k):
                    # one 512-column group of K (g<4) or Q (g<2) for head pair p
                    sl = slice(g * 512, (g + 1) * 512)
                    col = E + p * 128 if is_k else p * 128
                    dst = kTt[p] if is_k else qTt[p]
                    bcol = EC + p if is_k else p
                    ps = ps_sc.tile([128, TOK], F32, tag="sc", name="ps_kq")
                    for kp in range(ECP):
                        w = pair(wq8p[kp])[:, :, col:col + 128]
                        nc.tensor.matmul(ps[:, 0:512], w, pair(xap[kp])[:, :, sl],
                                         start=(kp == 0), stop=(kp == ECP - 1),
                                         perf_mode=PM.DoubleRow)
                    if is_k:
                        # ACT: identity(ps/WS + b) -- keeps DVE free during startup
                        nc.scalar.activation(dst[:, sl], ps[:, 0:512], AF.Identity,
                                             bias=bqkv[:, bcol:bcol + 1], scale=RWS)
                    else:
                        nc.vector.tensor_scalar(dst[:, sl], ps[:, 0:512], RWS,
                                                bqkv[:, bcol:bcol + 1],
                                                op0=OP.mult, op1=OP.add)

                def kq_chunk(p):
                    for g in range(4):
                        kq_group(p, g, True)
                    for g in range(2):
                        kq_group(p, g, False)

                def v_chunk(t):
                    ps = ps_sc.tile([128, TOK], F32, tag="sc", name="ps_v")
                    for kp in range(ECP):
                        nc.tensor.matmul(
                            ps[:, 0:512], pair(xap[kp])[:, :, t * 128:(t + 1) * 128],
                            pair(wq8p[kp])[:, :, 2 * E:3 * E],
                            start=(kp == 0), stop=(kp == ECP - 1),
                            perf_mode=PM.DoubleRow)
                    # V bias is folded into bo2d on the host (y += Wo @ bv), so
                    # the readout is a pure scaled copy -> ACT can do it
                    vr = vt8[t // 2][:].rearrange("p (i h c) -> p i h c", i=2, c=66)
                    nc.scalar.activation(
                        vr[:, t % 2, :, 0:64],
                        ps[:, 0:512].rearrange("t (h c) -> t h c", c=64),
                        AF.Copy, scale=RWS)

                def normalize(h, pso):
                    # 1/rowsum (psum row 64); replicate across partitions on gpsimd
                    hp, ro = h // 2, (h % 2) * 64
                    rr = pa.tile([1, TOK], F32R, tag="rr", bufs=2, name="rr")
                    with nc.allow_low_precision(reason="softmax denom rounded to f32r"):
                        nc.vector.reciprocal(rr[:], pso[64:65, :])
                    rsb = pa.tile([64, TOK], F32R, tag="rsb", bufs=2, name="rsb")
                    nc.gpsimd.partition_broadcast(rsb[:], rr[:])
                    nc.vector.tensor_mul(oT[hp][ro:ro + 64, :], pso[0:64, :],
                                         rsb[:].bitcast(F32))

                for p in range(EC):
                    kq_chunk(p)

                def s_exp(h, t, ex, v_inline=False):
                    # scores for key chunk t + exp into ex[:, t%2, :] (fp8)
                    hp, ro = h // 2, (h % 2) * 64
                    if v_inline:
                        v_chunk(t)
                    pssc = ps_sc.tile([128, TOK], F32, tag="sc", name="ps_sc")
                    for g in range(2):
                        sl = slice(g * 512, (g + 1) * 512)
                        nc.tensor.matmul(
                            pssc[:, sl],
                            kTt[hp][ro:ro + 64, t * 128:(t + 1) * 128],
                            qTt[hp][ro:ro + 64, sl],
                            start=True, stop=True)
                    dst = pair(ex)[:, t % 2, :]
                    kind = (EXPAT01 if h < 2 else EXPAT)[t]
                    if kind == "A":
                        nc.scalar.activation(dst, pssc[:], AF.Exp, scale=0.125)
                    else:
                        with nc.allow_low_precision(reason="schraudolph exp to fp8"):
                            nc.vector.tensor_scalar(dst.bitcast(U8), pssc[:], EXP_A, EXP_B,
                                                    op0=OP.mult, op1=OP.add)

                def av_pair(h, tp, pso, ex):
                    vv = pair(vt8[tp])[:, :, h * 66:h * 66 + 65]
                    exr = pair(ex)
                    for g in range(2):
                        sl = slice(g * 512, (g + 1) * 512)
                        nc.tensor.matmul(pso[:, sl], vv, exr[:, :, sl],
                                         start=(tp == 0), stop=(tp == KP - 1),
                                         perf_mode=PM.DoubleRow)

                # heads 0+1 share one interleaved key loop with inline V compute
                pso0 = ps_o.tile([65, TOK], F32, tag="pso", name="ps_av")
                pso1 = ps_o.tile([65, TOK], F32, tag="pso", name="ps_av")
                ex0 = pa.tile([128, 2 * TOK], FP8, tag="ex", bufs=4, name="ex")
                s_exp(0, 0, ex0, v_inline=True)
                s_exp(0, 1, ex0, v_inline=True)
                ex1 = pa.tile([128, 2 * TOK], FP8, tag="ex", bufs=4, name="ex")
                s_exp(1, 0, ex1)
                s_exp(1, 1, ex1)
                for tp in range(KP):
                    if tp + 1 < KP:
                        nx0 = pa.tile([128, 2 * TOK], FP8, tag="ex", bufs=4, name="ex")
                        s_exp(0, 2 * tp + 2, nx0, v_inline=True)
                        s_exp(0, 2 * tp + 3, nx0, v_inline=True)
                        nx1 = pa.tile([128, 2 * TOK], FP8, tag="ex", bufs=4, name="ex")
                        s_exp(1, 2 * tp + 2, nx1)
                        s_exp(1, 2 * tp + 3, nx1)
                    else:
                        nx0 = nx1 = None
                    av_pair(0, tp, pso0, ex0)
                    av_pair(1, tp, pso1, ex1)
                    ex0, ex1 = nx0, nx1
                normalize(0, pso0)
                normalize(1, pso1)
                # post-phase weights (DMA is idle during attention)
                for k in range(EC):
                    nc.sync.dma_start(woT[k][:], wo_in[k * 128:(k + 1) * 128, :])
                for kp in range(ECP):
                    nc.sync.dma_start(w18p[kp][:], w18_in[kp * 128:(kp + 1) * 128, :])
                for kp in range(HIDP):
                    nc.sync.dma_start(w28p[kp][:], w28_in[kp * 128:(kp + 1) * 128, :])
                nc.sync.dma_start(bo2d[:], bo_in[:])
                nc.sync.dma_start(b12d[:], b1_in[:])
                nc.sync.dma_start(g2d[:], g_in[:])
                nc.sync.dma_start(bb2d[:], bb_in[:])
                nc.sync.dma_start(bbb2[:], bbb2_in[:])
                nc.sync.dma_start(onr[:], ones_in[:].bitcast(F32R))
                nc.sync.dma_start(onc[:], onesc_in[:].bitcast(F32R))

                for h in range(2, H):
                    pso = ps_o.tile([65, TOK], F32, tag="pso", name="ps_av")
                    # software-pipelined: scores/exp of pair tp+1 are emitted
                    # BEFORE av(tp) so the in-order PE never waits on an exp
                    ex_t = pa.tile([128, 2 * TOK], FP8, tag="ex", bufs=4, name="ex")
                    s_exp(h, 0, ex_t)
                    s_exp(h, 1, ex_t)
                    for tp in range(KP):
                        if tp + 1 < KP:
                            ex_n = pa.tile([128, 2 * TOK], FP8, tag="ex", bufs=4, name="ex")
                            s_exp(h, 2 * tp + 2, ex_n)
                            s_exp(h, 2 * tp + 3, ex_n)
                        else:
                            ex_n = None
                        av_pair(h, tp, pso, ex_t)
                        ex_t = ex_n
                    normalize(h, pso)
                    if h == 6:
                        # x+pe fp32 residual, late, on the idle gpsimd
                        for k in range(EC):
                            nc.gpsimd.tensor_scalar_add(xw[k][:], xrf[k][:], pe2[:, k:k + 1])

            # ================= Stage P: out-proj + LN1 + FFN + LN2 =================
            with tc.tile_pool(name="pp", bufs=1) as pp, \
                 tc.tile_pool(name="ps_mm", bufs=2, space="PSUM") as ps_mm, \
                 tc.tile_pool(name="ps_ln", bufs=1, space="PSUM") as ps_ln:

                xres = [pp.tile([128, TOK], F32R, tag=f"xres{m}", name=f"xres{m}") for m in range(EC)]
                x1bb = [pp.tile([128, TOK], BF16, tag=f"x1b{m}", name=f"x1b{m}") for m in range(EC)]
                x18 = [pp.tile([128, 2 * TOK], FP8, tag=f"x18{m}", name=f"x18{m}") for m in range(ECP)]
                yt = [pp.tile([128, TOK], F32, tag=f"yt{m}", name=f"yt{m}") for m in range(EC)]
                hT8 = [pp.tile([128, 2 * TOK], FP8, tag=f"hT{m}", name=f"hT{m}") for m in range(HIDP)]

                _ln = {}

                def ln_stats(src, gh):
                    """Stats + per-token scalar chain for one column half."""
                    sl = slice(gh * 512, (gh + 1) * 512)
                    sqs = []
                    for k in range(EC):
                        sq = pp.tile([128, 512], F32R, tag="sq", bufs=2, name="sq")
                        eng = nc.vector if k % 2 == 0 else nc.gpsimd
                        with nc.allow_low_precision(reason="LN variance in f32r"):
                            eng.tensor_mul(sq[:], src[k][:, sl], src[k][:, sl])
                        sqs.append(sq)
                    pss = ps_ln.tile([1, 512], F32, tag="pss", name="ps_mean")
                    for k in range(EC):
                        nc.tensor.matmul(pss[:], onc[:], src[k][:, sl],
                                         start=(k == 0), stop=(k == EC - 1))
                    pss2 = ps_ln.tile([1, 512], F32, tag="pss2", name="ps_var")
                    for k in range(EC):
                        nc.tensor.matmul(pss2[:], onc[:], sqs[k][:],
                                         start=(k == 0), stop=(k == EC - 1))
                    # ones_col carries 1/E, so pss/pss2 are already E[x], E[x^2]
                    rows = pp.tile([1, 2 * 512], F32, tag="lnrows", bufs=2, name="lnrows")
                    rowsr = pp.tile([1, 2 * 512], F32R, tag="lnrowsr", bufs=2, name="lnrowsr")
                    mu2 = rows[0:1, 0:512]
                    rec = rows[0:1, 512:1024]
                    mur = rowsr[0:1, 0:512]
                    rsq = rowsr[0:1, 512:1024]
                    nc.vector.tensor_copy(mur, pss[:])
                    nc.scalar.activation(mu2, pss[:], AF.Square)
                    nc.vector.scalar_tensor_tensor(rec, pss2[:], 1.0, mu2,
                                                   op0=OP.mult, op1=OP.subtract)
                    nc.vector.tensor_scalar_add(rec, rec, 1e-5)
                    nc.vector.reciprocal(rec, rec)
                    nc.scalar.activation(rsq, rec, AF.Sqrt)
                    _ln[gh] = (mur, rsq)

                def ln_finish(src, dst, gh, dma=False, fp8_dst=None, fold_b2=False):
                    """Replicate + normalize one column half. Each feature chunk's
                    3-op chain runs on ONE engine, alternating DVE/gpsimd. With
                    fp8_dst also writes the fp8 pair copy (on the other engine)."""
                    sl = slice(gh * 512, (gh + 1) * 512)
                    mur, rsq = _ln[gh]
                    psm = ps_ln.tile([128, TOK], F32, tag="psm", name="ps_lnrep")
                    nc.tensor.matmul(psm[:, 0:512], onr[:], mur, start=True, stop=True)
                    nc.tensor.matmul(psm[:, 512:1024], onr[:], rsq, start=True, stop=True)
                    msb = pp.tile([128, 512], F32, tag="lnmsb", bufs=1, name="lnmsb")
                    rsb = pp.tile([128, 512], F32, tag="lnrsb", bufs=1, name="lnrsb")
                    nc.vector.tensor_copy(msb[:], psm[:, 0:512])
                    nc.vector.tensor_copy(rsb[:], psm[:, 512:1024])
                    bias = bbb2 if fold_b2 else bb2d
                    for k in range(EC):
                        eng = nc.vector if k % 2 == 0 else nc.gpsimd
                        oth = nc.gpsimd if k % 2 == 0 else nc.vector
                        t1 = pp.tile([128, 512], F32, tag="t1", bufs=2, name="t1")
                        eng.tensor_sub(t1[:], src[k][:, sl].bitcast(F32), msb[:])
                        t2 = pp.tile([128, 512], F32, tag="t2", bufs=2, name="t2")
                        eng.tensor_mul(t2[:], t1[:], rsb[:])
                        eng.tensor_scalar(dst[k][:, sl], t2[:], g2d[:, k:k + 1],
                                          bias[:, k:k + 1], op0=OP.mult, op1=OP.add)
                        if fp8_dst is not None:
                            with nc.allow_low_precision(reason="fp8 copy for DR matmul"):
                                oth.tensor_scalar(
                                    pair(fp8_dst[k // 2])[:, k % 2, sl], t2[:],
                                    g2d[:, k:k + 1], bb2d[:, k:k + 1],
                                    op0=OP.mult, op1=OP.add)
                        if dma:
                            nc.sync.dma_start(yT_out[k * 128:(k + 1) * 128, sl], dst[k][:, sl])

                def proj(g):
                    sl = slice(g * 512, (g + 1) * 512)
                    for m in range(EC):
                        pst = ps_mm.tile([128, 512], F32, tag="mm", name="ps_proj")
                        for k in range(EC):
                            nc.tensor.matmul(pst[:], woT[k][:, m * 128:(m + 1) * 128],
                                             oT[k][:, sl], start=(k == 0), stop=(k == EC - 1))
                        nc.vector.scalar_tensor_tensor(
                            xres[m][:, sl], pst[:], bo2d[:, m:m + 1], xw[m][:, sl],
                            op0=OP.add, op1=OP.add)

                def ffn1(g):
                    sl = slice(g * 512, (g + 1) * 512)
                    for m in range(HIDC):
                        psf = ps_mm.tile([128, 512], F32, tag="mm", name="ps_f1")
                        for kp in range(ECP):
                            nc.tensor.matmul(psf[:], pair(w18p[kp])[:, :, m * 128:(m + 1) * 128],
                                             pair(x18[kp])[:, :, sl],
                                             start=(kp == 0), stop=(kp == ECP - 1),
                                             perf_mode=PM.DoubleRow)
                        nc.scalar.activation(pair(hT8[m // 2])[:, m % 2, sl], psf[:],
                                             AF.Gelu, bias=b12d[:, m:m + 1], scale=RWS)

                def ffn2(g):
                    # b2 is pre-folded into x1bb's bias; readout is one STT
                    sl = slice(g * 512, (g + 1) * 512)
                    for m in range(EC):
                        psg = ps_mm.tile([128, 512], F32, tag="mm", name="ps_f2")
                        for kp in range(HIDP):
                            nc.tensor.matmul(psg[:], pair(w28p[kp])[:, :, m * 128:(m + 1) * 128],
                                             pair(hT8[kp])[:, :, sl],
                                             start=(kp == 0), stop=(kp == HIDP - 1),
                                             perf_mode=PM.DoubleRow)
                        nc.vector.scalar_tensor_tensor(
                            xres[m][:, sl], psg[:], RWS, x1bb[m][:, sl],
                            op0=OP.mult, op1=OP.add)

                proj(0)
                ln_stats(xres, 0)
                proj(1)
                ln_finish(xres, x1bb, 0, fp8_dst=x18, fold_b2=True)
                ln_stats(xres, 1)
                ffn1(0)
                ln_finish(xres, x1bb, 1, fp8_dst=x18, fold_b2=True)
                ffn1(1)
                ffn2(0)
                ln_stats(xres, 0)
                ffn2(1)
                ln_finish(xres, yt, 0, dma=True)
                ln_stats(xres, 1)
                ln_finish(xres, yt, 1, dma=True)

    nc.compile()
    _BUILD_CACHE["nc"] = nc
    return nc


def _pos_encoding_np(S, Emb):
    t = np.arange(S, dtype=np.float32)[:, None]
    i = np.arange(Emb, dtype=np.float32)[None, :]
    even = np.sin((t + 1.0) * np.power(np.float32(10000.0), -i / Emb))
    odd = np.cos((t + 1.0) * np.power(np.float32(10000.0), -(i + 1.0) / Emb))
    return np.where(np.arange(Emb)[None, :] % 2 == 0, even, odd).astype(np.float32)


def _pack_pairs(wT, fp8):
    """(Kc*128, N) -> (Kc/2*128, 2*N): row (kp*128+p), col (i*N+c) = wT[(2kp+i)*128+p, c]."""
    K, N = wT.shape
    kc = K // 128
    return np.ascontiguousarray(
        wT.reshape(kc // 2, 2, 128, N).transpose(0, 2, 1, 3).reshape(kc // 2 * 128, 2 * N)
    ).astype(fp8)


def prepare_in_maps(x, in_proj_w, in_proj_b, out_w, out_b, w1, b1, w2, b2, ln_g, ln_b):
    import ml_dtypes
    bf16 = ml_dtypes.bfloat16
    fp8 = ml_dtypes.float8_e4m3
    pe = _pos_encoding_np(B, E)                      # (B, E)
    wq, wk, wv = in_proj_w[:E], in_proj_w[E:2 * E], in_proj_w[2 * E:]
    wqkvT = np.concatenate([wq.T, wk.T, wv.T], axis=1)   # (E, 3E)
    shared = {
        "wqkv8": _pack_pairs(wqkvT * WS, fp8),
        "woT": np.ascontiguousarray(out_w.T).astype(bf16),
        "w18": _pack_pairs(w1.T * WS, fp8),
        "w28": _pack_pairs(w2.T * WS, fp8),
        "bqkv2d": np.ascontiguousarray(in_proj_b.reshape(3 * EC, 128).T),
        # v bias folded through the out-projection: y = Wo(o0 + bv) + bo
        "bo2d": np.ascontiguousarray((out_b + out_w @ in_proj_b[2 * E:]).reshape(EC, 128).T),
        "b1_2d": np.ascontiguousarray(b1.reshape(HIDC, 128).T),
        "g2d": np.ascontiguousarray(ln_g.reshape(EC, 128).T),
        "bb2d": np.ascontiguousarray(ln_b.reshape(EC, 128).T),
        "bb2d_b2": np.ascontiguousarray((ln_b + b2).reshape(EC, 128).T),
        "ones_row": np.ones((1, 128), np.float32),
        "ones_col": np.full((128, 1), 1.0 / E, np.float32),  # LN stats: mean in one matmul
    }
    in_maps = []
    for c in range(NCORES):
        b, s = c // 2, c % 2
        xb = x[:, b, :]                              # (L, E)
        # rotate so own tokens are first: [own 1024 | other 1024]
        xrot = np.concatenate([xb[s * TOK:(s + 1) * TOK], xb[(1 - s) * TOK:(2 - s) * TOK]], axis=0)
        m = dict(shared)
        m["xT"] = np.ascontiguousarray(xrot.T).astype(bf16)   # (E, L)
        m["xTf"] = np.ascontiguousarray(xb[s * TOK:(s + 1) * TOK].T)  # (E, TOK) fp32
        m["pe2d"] = np.ascontiguousarray(pe[b].reshape(EC, 128).T)
        in_maps.append(m)
    return in_maps


def assemble_output(results):
    y = np.empty((L, B, E), np.float32)
    for c in range(NCORES):
        b, s = c // 2, c % 2
        y[s * TOK:(s + 1) * TOK, b, :] = results[c]["yT"].T
    return y


def kernel(**inputs):
    inputs = {k: np.asarray(v, dtype=np.float32) for k, v in inputs.items()}
    nc = build_encoder()
    in_maps = prepare_in_maps(**inputs)
    res = run_bass_kernel_spmd(nc, in_maps, core_ids=list(range(NCORES)))
    return assemble_output(res.results)


# revision 11
# speedup vs baseline: 1.1538x; 1.0059x over previous
"""Trainium2 Bass kernel for nn_Encoder_88691074663154 (dense transformer encoder layer).

Strategy v4: batch x sequence sharding (core c = (b, s): batch c//2, sequence
half c%2), ZERO collectives; K/V recomputed per core for the full 2048 keys.
The host rotates each core's token order so its own 1024 query tokens are
always columns 0:1023 -> one uniform SPMD program.

Perf structure:
- QKV / AV / FFN1 / FFN2 matmuls run in fp8e4 with MatmulPerfMode.DoubleRow
  ([128, 2, N] pairs of contraction tiles, 0.5 cycles/col = 4x bf16).
  Weights are scaled by 64 on the host (fp8e4 subnormal floor); the 1/64 is
  folded into the psum readouts. Scores and out-proj stay bf16; LN f32r.
- The host pre-adds the positional encoding and ships x as fp8 pairs (xa8)
  plus an f32 residual copy (xw) - no on-chip x+pe ops at all.
- softmax exp splits across ACT (table exp, fp8 out) and DVE (Schraudolph:
  u8 = round(log2(e)*s + b) IS the fp8e4 bit pattern of exp(s/8); the ~3%
  error cancels through softmax normalization - verified end to end).
- Heads run in PAIRS with interleaved key loops: each scores-psum ring slot
  then has two heads of work between reuses, hiding the scores->exp->AV
  round-trip latency that otherwise serializes the attention loop.
- V bias is folded through the out-projection (bo += Wo @ bv) so the V
  readout is a pure scaled copy; b2 is folded into the LN1 bias of the bf16
  residual copy so the FFN2 readout stays one STT op.
"""
import os
import sys

sys.path.insert(0, "/opt/trn_rl_repo")

import numpy as np

import concourse.bacc as bacc
import concourse.mybir as mybir
import concourse.tile as tile
from concourse.bass_utils import run_bass_kernel_spmd

F32 = mybir.dt.float32
F32R = mybir.dt.float32r
BF16 = mybir.dt.bfloat16
FP8 = mybir.dt.float8e4
U8 = mybir.dt.uint8
AF = mybir.ActivationFunctionType
OP = mybir.AluOpType
PM = mybir.MatmulPerfMode

L, B, E, H, HD, HID = 2048, 4, 512, 8, 64, 2048
NCORES = 8
TOK = 1024                # own query tokens per core
EC = E // 128             # 4 feature chunks
ECP = EC // 2             # 2 feature chunk pairs
HIDC = HID // 128         # 16 hidden chunks
HIDP = HIDC // 2          # 8 hidden chunk pairs
KT = L // 128             # 16 key chunks
KP = KT // 2              # 8 key chunk pairs

WS = 64.0                 # fp8 weight scale
RWS = 1.0 / WS
# Schraudolph exp -> fp8e4 bits: u8 = round(EXP_A * s + EXP_B) where s is the
# raw qk psum (the /8 softmax scale is folded into EXP_A).
EXP_A = 12102203.161561485 / 2**20 / 8.0
EXP_B = 55.650580406188965 - 0.12

# exp engine schedule per head: A=ACT table exp, D=DVE schraudolph
EXPAT01 = "ADADADADADADADAA"   # heads 0/1: A9 D7
EXPAT = "ADADADADADAADAAA"     # heads 2-7: A10 D6

_BUILD_CACHE = {}


def build_encoder():
    if "nc" in _BUILD_CACHE:
        return _BUILD_CACHE["nc"]
    nc = bacc.Bacc(None, num_devices=NCORES)

    # ---- DRAM parameters (per core) ----
    xa8_in = nc.declare_dram_parameter("xa8", [ECP * 128, 2 * L], FP8, isOutput=False)
    xw_in = nc.declare_dram_parameter("xw", [E, TOK], F32, isOutput=False)
    wqkv8_in = nc.declare_dram_parameter("wqkv8", [ECP * 128, 2 * 3 * E], FP8, isOutput=False)
    wo_in = nc.declare_dram_parameter("woT", [E, E], BF16, isOutput=False)
    w18_in = nc.declare_dram_parameter("w18", [ECP * 128, 2 * HID], FP8, isOutput=False)
    w28_in = nc.declare_dram_parameter("w28", [HIDP * 128, 2 * E], FP8, isOutput=False)
    bqkv_in = nc.declare_dram_parameter("bqkv2d", [128, 3 * EC], F32, isOutput=False)
    bo_in = nc.declare_dram_parameter("bo2d", [128, EC], F32, isOutput=False)
    b1_in = nc.declare_dram_parameter("b1_2d", [128, HIDC], F32, isOutput=False)
    g_in = nc.declare_dram_parameter("g2d", [128, EC], F32, isOutput=False)
    bb_in = nc.declare_dram_parameter("bb2d", [128, EC], F32, isOutput=False)
    bbb2_in = nc.declare_dram_parameter("bb2d_b2", [128, EC], F32, isOutput=False)
    ones_in = nc.declare_dram_parameter("ones_row", [1, 128], F32, isOutput=False)
    onesc_in = nc.declare_dram_parameter("ones_col", [128, 1], F32, isOutput=False)
    yT_out = nc.declare_dram_parameter("yT", [E, TOK], F32, isOutput=True)

    with tile.TileContext(nc) as tc:
        from contextlib import ExitStack
        with ExitStack() as ctx:
            pers = ctx.enter_context(tc.tile_pool(name="pers", bufs=1))

            # ---- persistent tiles ----
            onr = pers.tile([1, 128], F32R, tag="onr")
            onc = pers.tile([128, 1], F32R, tag="onc")
            bqkv = pers.tile([128, 3 * EC], F32, tag="bqkv")
            bo2d = pers.tile([128, EC], F32, tag="bo2d")
            b12d = pers.tile([128, HIDC], F32, tag="b12d")
            g2d = pers.tile([128, EC], F32, tag="g2d")
            bb2d = pers.tile([128, EC], F32, tag="bb2d")
            bbb2 = pers.tile([128, EC], F32, tag="bbb2")

            xw = [pers.tile([128, TOK], F32, tag=f"xw{k}", name=f"xw{k}") for k in range(EC)]
            kTt = [pers.tile([128, L], BF16, tag=f"kT{p}", name=f"kT{p}") for p in range(EC)]
            qTt = [pers.tile([128, TOK], BF16, tag=f"qT{p}", name=f"qT{p}") for p in range(EC)]
            # V pairs: [128, 2, H*66] fp8: per head 64 values + ones col (for the
            # softmax denominator) + 1 pad col (dual-fp8 Ldweights needs even /
            # 16B-aligned strides)
            vt8 = [pers.tile([128, 2 * H * 66], FP8, tag=f"vt{t}", name=f"vt{t}") for t in range(KP)]
            woT = [pers.tile([128, E], BF16, tag=f"woT{k}", name=f"woT{k}") for k in range(EC)]
            w18p = [pers.tile([128, 2 * HID], FP8, tag=f"w18{k}", name=f"w18{k}") for k in range(ECP)]
            w28p = [pers.tile([128, 2 * E], FP8, tag=f"w28{k}", name=f"w28{k}") for k in range(HIDP)]
            oT = [pers.tile([128, TOK], BF16, tag=f"oT{p}", name=f"oT{p}") for p in range(EC)]

            def pair(t):
                return t[:].rearrange("p (i x) -> p i x", i=2)

            # ========== Stage Q+A: QKV interleaved with attention ==========
            with tc.tile_pool(name="pq", bufs=1) as pq, \
                 tc.tile_pool(name="pa", bufs=1) as pa, \
                 tc.tile_pool(name="ps_sc", bufs=2, space="PSUM") as ps_sc, \
                 tc.tile_pool(name="ps_o", bufs=2, space="PSUM") as ps_o:
                wq8p = [pq.tile([128, 2 * 3 * E], FP8, tag=f"wq8{k}", name=f"wq8{k}") for k in range(ECP)]
                xap = [pq.tile([128, 2 * L], FP8, tag=f"xa{k}", name=f"xa{k}") for k in range(ECP)]

                for kp in range(ECP):
                    nc.sync.dma_start(xap[kp][:], xa8_in[kp * 128:(kp + 1) * 128, :])
                    nc.sync.dma_start(wq8p[kp][:], wqkv8_in[kp * 128:(kp + 1) * 128, :])
                nc.sync.dma_start(bqkv[:], bqkv_in[:])
                # ones columns of V (written once; disjoint from the value cols)
                for tp in range(KP):
                    vr = vt8[tp][:].rearrange("p (i h c) -> p i h c", i=2, c=66)
                    nc.vector.memset(vr[:, :, :, 64:65], 1.0)

                def kq_group2(p, gg, is_k):
                    # one 1024-column double-group of K (gg<2) or Q (gg=0) for
                    # head pair p; one batched readout
                    col = E + p * 128 if is_k else p * 128
                    dst = kTt[p] if is_k else qTt[p]
                    bcol = EC + p if is_k else p
                    ps = ps_sc.tile([128, TOK], F32, tag="sc", name="ps_kq")
                    for g in (0, 1):
                        sl = slice((2 * gg + g) * 512, (2 * gg + g + 1) * 512) if is_k \
                            else slice(g * 512, (g + 1) * 512)
                        psl = slice(g * 512, (g + 1) * 512)
                        for kp in range(ECP):
                            w = pair(wq8p[kp])[:, :, col:col + 128]
                            nc.tensor.matmul(ps[:, psl], w, pair(xap[kp])[:, :, sl],
                                             start=(kp == 0), stop=(kp == ECP - 1),
                                             perf_mode=PM.DoubleRow)
                    dsl = slice(2 * gg * 512, (2 * gg + 2) * 512) if is_k else slice(0, TOK)
                    if is_k:
                        # ACT: identity(ps/WS + b)
                        nc.scalar.activation(dst[:, dsl], ps[:], AF.Identity,
                                             bias=bqkv[:, bcol:bcol + 1], scale=RWS)
                    else:
                        nc.vector.tensor_scalar(dst[:, dsl], ps[:], RWS,
                                                bqkv[:, bcol:bcol + 1],
                                                op0=OP.mult, op1=OP.add)

                def kq_chunk(p):
                    kq_group2(p, 0, True)
                    kq_group2(p, 1, True)
                    kq_group2(p, 0, False)

                def v_chunk(t):
                    ps = ps_sc.tile([128, TOK], F32, tag="sc", name="ps_v")
                    for kp in range(ECP):
                        nc.tensor.matmul(
                            ps[:, 0:512], pair(xap[kp])[:, :, t * 128:(t + 1) * 128],
                            pair(wq8p[kp])[:, :, 2 * E:3 * E],
                            start=(kp == 0), stop=(kp == ECP - 1),
                            perf_mode=PM.DoubleRow)
                    # V bias folded into bo2d on the host -> pure scaled copy
                    vr = vt8[t // 2][:].rearrange("p (i h c) -> p i h c", i=2, c=66)
                    dst = vr[:, t % 2, :, 0:64]
                    src = ps[:, 0:512].rearrange("t (h c) -> t h c", c=64)
                    if t % 2 == 0:
                        with nc.allow_low_precision(reason="fp8 V for DR matmul"):
                            nc.vector.tensor_scalar_mul(dst, src, RWS)
                    else:
                        nc.scalar.activation(dst, src, AF.Copy, scale=RWS)

                def normalize(h, pso):
                    # 1/rowsum (psum row 64); replicate across partitions on gpsimd
                    hp, ro = h // 2, (h % 2) * 64
                    rr = pa.tile([1, TOK], F32R, tag="rr", bufs=2, name="rr")
                    with nc.allow_low_precision(reason="softmax denom rounded to f32r"):
                        nc.vector.reciprocal(rr[:], pso[64:65, :])
                    rsb = pa.tile([64, TOK], F32R, tag="rsb", bufs=2, name="rsb")
                    nc.gpsimd.partition_broadcast(rsb[:], rr[:])
                    nc.vector.tensor_mul(oT[hp][ro:ro + 64, :], pso[0:64, :],
                                         rsb[:].bitcast(F32))

                for p in range(EC):
                    kq_chunk(p)

                def s_exp(h, t, ex, v_inline=False):
                    # scores for key chunk t + exp into ex[:, t%2, :] (fp8)
                    hp, ro = h // 2, (h % 2) * 64
                    if v_inline:
                        v_chunk(t)
                    pssc = ps_sc.tile([128, TOK], F32, tag="sc", name="ps_sc")
                    for g in range(2):
                        sl = slice(g * 512, (g + 1) * 512)
                        nc.tensor.matmul(
                            pssc[:, sl],
                            kTt[hp][ro:ro + 64, t * 128:(t + 1) * 128],
                            qTt[hp][ro:ro + 64, sl],
                            start=True, stop=True)
                    dst = pair(ex)[:, t % 2, :]
                    kind = (EXPAT01 if h < 2 else EXPAT)[t]
                    if kind == "A":
                        nc.scalar.activation(dst, pssc[:], AF.Exp, scale=0.125)
                    else:
                        with nc.allow_low_precision(reason="schraudolph exp to fp8"):
                            nc.vector.tensor_scalar(dst.bitcast(U8), pssc[:], EXP_A, EXP_B,
                                                    op0=OP.mult, op1=OP.add)

                def av_pair(h, tp, pso, ex):
                    vv = pair(vt8[tp])[:, :, h * 66:h * 66 + 65]
                    exr = pair(ex)
                    for g in range(2):
                        sl = slice(g * 512, (g + 1) * 512)
                        nc.tensor.matmul(pso[:, sl], vv, exr[:, :, sl],
                                         start=(tp == 0), stop=(tp == KP - 1),
                                         perf_mode=PM.DoubleRow)

                def new_ex():
                    return pa.tile([128, 2 * TOK], FP8, tag="ex", bufs=4, name="ex")

                def head_pair(h0, h1, v_inline=False):
                    # two heads share one interleaved key loop: the scores-psum
                    # ring slots get 2 heads of work between reuses, hiding the
                    # scores->exp->AV round trip
                    pso0 = ps_o.tile([65, TOK], F32, tag="pso", name="ps_av")
                    pso1 = ps_o.tile([65, TOK], F32, tag="pso", name="ps_av")
                    ex0 = new_ex()
                    s_exp(h0, 0, ex0, v_inline=v_inline)
                    s_exp(h0, 1, ex0, v_inline=v_inline)
                    ex1 = new_ex()
                    s_exp(h1, 0, ex1)
                    s_exp(h1, 1, ex1)
                    for tp in range(KP):
                        if tp + 1 < KP:
                            nx0 = new_ex()
                            s_exp(h0, 2 * tp + 2, nx0, v_inline=v_inline)
                            s_exp(h0, 2 * tp + 3, nx0, v_inline=v_inline)
                            nx1 = new_ex()
                            s_exp(h1, 2 * tp + 2, nx1)
                            s_exp(h1, 2 * tp + 3, nx1)
                        else:
                            nx0 = nx1 = None
                        av_pair(h0, tp, pso0, ex0)
                        if tp == KP - 1:
                            normalize(h0, pso0)
                        av_pair(h1, tp, pso1, ex1)
                        if tp == KP - 1:
                            normalize(h1, pso1)
                        ex0, ex1 = nx0, nx1

                head_pair(0, 1, v_inline=True)
                # post-phase weights + residual (DMA is idle during attention)
                for k in range(EC):
                    nc.sync.dma_start(woT[k][:], wo_in[k * 128:(k + 1) * 128, :])
                    nc.sync.dma_start(xw[k][:], xw_in[k * 128:(k + 1) * 128, :])
                for kp in range(ECP):
                    nc.sync.dma_start(w18p[kp][:], w18_in[kp * 128:(kp + 1) * 128, :])
                for kp in range(HIDP):
                    nc.sync.dma_start(w28p[kp][:], w28_in[kp * 128:(kp + 1) * 128, :])
                nc.sync.dma_start(bo2d[:], bo_in[:])
                nc.sync.dma_start(b12d[:], b1_in[:])
                nc.sync.dma_start(g2d[:], g_in[:])
                nc.sync.dma_start(bb2d[:], bb_in[:])
                nc.sync.dma_start(bbb2[:], bbb2_in[:])
                nc.sync.dma_start(onr[:], ones_in[:].bitcast(F32R))
                nc.sync.dma_start(onc[:], onesc_in[:].bitcast(F32R))

                head_pair(2, 3)
                head_pair(4, 5)
                head_pair(6, 7)

            # ================= Stage P: out-proj + LN1 + FFN + LN2 =================
            with tc.tile_pool(name="pp", bufs=1) as pp, \
                 tc.tile_pool(name="ps_mm", bufs=2, space="PSUM") as ps_mm, \
                 tc.tile_pool(name="ps_ln", bufs=1, space="PSUM") as ps_ln:

                xres = [pp.tile([128, TOK], F32R, tag=f"xres{m}", name=f"xres{m}") for m in range(EC)]
                x1bb = [pp.tile([128, TOK], BF16, tag=f"x1b{m}", name=f"x1b{m}") for m in range(EC)]
                x18 = [pp.tile([128, 2 * TOK], FP8, tag=f"x18{m}", name=f"x18{m}") for m in range(ECP)]
                yt = [pp.tile([128, TOK], F32, tag=f"yt{m}", name=f"yt{m}") for m in range(EC)]
                hT8 = [pp.tile([128, 2 * TOK], FP8, tag=f"hT{m}", name=f"hT{m}") for m in range(HIDP)]

                _ln = {}

                def ln_stats(src, gh):
                    """Stats + per-token scalar chain for one column half."""
                    sl = slice(gh * 512, (gh + 1) * 512)
                    sqs = []
                    for k in range(EC):
                        sq = pp.tile([128, 512], F32R, tag="sq", bufs=2, name="sq")
                        eng = nc.vector if k % 2 == 0 else nc.gpsimd
                        with nc.allow_low_precision(reason="LN variance in f32r"):
                            eng.tensor_mul(sq[:], src[k][:, sl], src[k][:, sl])
                        sqs.append(sq)
                    pss = ps_ln.tile([1, 512], F32, tag="pss", name="ps_mean")
                    for k in range(EC):
                        nc.tensor.matmul(pss[:], onc[:], src[k][:, sl],
                                         start=(k == 0), stop=(k == EC - 1))
                    pss2 = ps_ln.tile([1, 512], F32, tag="pss2", name="ps_var")
                    for k in range(EC):
                        nc.tensor.matmul(pss2[:], onc[:], sqs[k][:],
                                         start=(k == 0), stop=(k == EC - 1))
                    # ones_col carries 1/E, so pss/pss2 are already E[x], E[x^2]
                    rows = pp.tile([1, 2 * 512], F32, tag="lnrows", bufs=2, name="lnrows")
                    rowsr = pp.tile([1, 2 * 512], F32R, tag="lnrowsr", bufs=2, name="lnrowsr")
                    mu2 = rows[0:1, 0:512]
                    rec = rows[0:1, 512:1024]
                    mur = rowsr[0:1, 0:512]
                    rsq = rowsr[0:1, 512:1024]
                    nc.vector.tensor_copy(mur, pss[:])
                    nc.scalar.activation(mu2, pss[:], AF.Square)
                    nc.vector.scalar_tensor_tensor(rec, pss2[:], 1.0, mu2,
                                                   op0=OP.mult, op1=OP.subtract)
                    nc.vector.tensor_scalar_add(rec, rec, 1e-5)
                    nc.vector.reciprocal(rec, rec)
                    nc.scalar.activation(rsq, rec, AF.Sqrt)
                    _ln[gh] = (mur, rsq)

                # per-chunk engine assignment for the LN normalize chains
                CHAIN = [None, None, None, None]
                DSTE = ["A", "D", "A", "D"]

                def ln_finish(src, dst, gh, dma=False, fp8_dst=None, fold_b2=False):
                    """Replicate + normalize one column half; chains spread over
                    gpsimd/DVE, affine writes over ACT/DVE."""
                    sl = slice(gh * 512, (gh + 1) * 512)
                    mur, rsq = _ln[gh]
                    psm = ps_ln.tile([128, TOK], F32, tag="psm", name="ps_lnrep")
                    nc.tensor.matmul(psm[:, 0:512], onr[:], mur, start=True, stop=True)
                    nc.tensor.matmul(psm[:, 512:1024], onr[:], rsq, start=True, stop=True)
                    msb = pp.tile([128, 512], F32, tag="lnmsb", bufs=1, name="lnmsb")
                    rsb = pp.tile([128, 512], F32, tag="lnrsb", bufs=1, name="lnrsb")
                    nc.vector.tensor_copy(msb[:], psm[:, 0:512])
                    nc.vector.tensor_copy(rsb[:], psm[:, 512:1024])
                    bias = bbb2 if fold_b2 else bb2d
                    for k in range(EC):
                        eng = nc.vector if k == 1 else nc.gpsimd
                        t1 = pp.tile([128, 512], F32, tag="t1", bufs=2, name="t1")
                        eng.tensor_sub(t1[:], src[k][:, sl].bitcast(F32), msb[:])
                        t2 = pp.tile([128, 512], F32, tag="t2", bufs=2, name="t2")
                        eng.tensor_mul(t2[:], t1[:], rsb[:])
                        if DSTE[k] == "A":
                            nc.scalar.activation(dst[k][:, sl], t2[:], AF.Identity,
                                                 bias=bias[:, k:k + 1], scale=g2d[:, k:k + 1])
                        else:
                            nc.vector.tensor_scalar(dst[k][:, sl], t2[:], g2d[:, k:k + 1],
                                                    bias[:, k:k + 1], op0=OP.mult, op1=OP.add)
                        if fp8_dst is not None:
                            with nc.allow_low_precision(reason="fp8 copy for DR matmul"):
                                nc.gpsimd.tensor_scalar(
                                    pair(fp8_dst[k // 2])[:, k % 2, sl], t2[:],
                                    g2d[:, k:k + 1], bb2d[:, k:k + 1],
                                    op0=OP.mult, op1=OP.add)
                        if dma:
                            nc.sync.dma_start(yT_out[k * 128:(k + 1) * 128, sl], dst[k][:, sl])

                def proj(g):
                    sl = slice(g * 512, (g + 1) * 512)
                    for m in range(EC):
                        pst = ps_mm.tile([128, 512], F32, tag="mm", name="ps_proj")
                        for k in range(EC):
                            nc.tensor.matmul(pst[:], woT[k][:, m * 128:(m + 1) * 128],
                                             oT[k][:, sl], start=(k == 0), stop=(k == EC - 1))
                        nc.vector.scalar_tensor_tensor(
                            xres[m][:, sl], pst[:], bo2d[:, m:m + 1], xw[m][:, sl],
                            op0=OP.add, op1=OP.add)

                def ffn1(g):
                    sl = slice(g * 512, (g + 1) * 512)
                    for m in range(HIDC):
                        psf = ps_mm.tile([128, 512], F32, tag="mm", name="ps_f1")
                        for kp in range(ECP):
                            nc.tensor.matmul(psf[:], pair(w18p[kp])[:, :, m * 128:(m + 1) * 128],
                                             pair(x18[kp])[:, :, sl],
                                             start=(kp == 0), stop=(kp == ECP - 1),
                                             perf_mode=PM.DoubleRow)
                        nc.scalar.activation(pair(hT8[m // 2])[:, m % 2, sl], psf[:],
                                             AF.Gelu, bias=b12d[:, m:m + 1], scale=RWS)

                def ffn2(g):
                    # b2 is pre-folded into x1bb's bias; readout is one STT
                    sl = slice(g * 512, (g + 1) * 512)
                    for m in range(EC):
                        psg = ps_mm.tile([128, 512], F32, tag="mm", name="ps_f2")
                        for kp in range(HIDP):
                            nc.tensor.matmul(psg[:], pair(w28p[kp])[:, :, m * 128:(m + 1) * 128],
                                             pair(hT8[kp])[:, :, sl],
                                             start=(kp == 0), stop=(kp == HIDP - 1),
                                             perf_mode=PM.DoubleRow)
                        nc.vector.scalar_tensor_tensor(
                            xres[m][:, sl], psg[:], RWS, x1bb[m][:, sl],
                            op0=OP.mult, op1=OP.add)

                proj(0)
                ln_stats(xres, 0)
                proj(1)
                ln_finish(xres, x1bb, 0, fp8_dst=x18, fold_b2=True)
                ln_stats(xres, 1)
                ffn1(0)
                ln_finish(xres, x1bb, 1, fp8_dst=x18, fold_b2=True)
                ffn1(1)
                ffn2(0)
                ln_stats(xres, 0)
                ffn2(1)
                ln_finish(xres, yt, 0, dma=True)
                ln_stats(xres, 1)
                ln_finish(xres, yt, 1, dma=True)

    nc.compile()
    _BUILD_CACHE["nc"] = nc
    return nc


def _pos_encoding_np(S, Emb):
    t = np.arange(S, dtype=np.float32)[:, None]
    i = np.arange(Emb, dtype=np.float32)[None, :]
    even = np.sin((t + 1.0) * np.power(np.float32(10000.0), -i / Emb))
    odd = np.cos((t + 1.0) * np.power(np.float32(10000.0), -(i + 1.0) / Emb))
    return np.where(np.arange(Emb)[None, :] % 2 == 0, even, odd).astype(np.float32)


def _pack_pairs(wT, fp8, scale=1.0):
    """(Kc*128, N) -> (Kc/2*128, 2*N): row (kp*128+p), col (i*N+c) = wT[(2kp+i)*128+p, c]."""
    K, N = wT.shape
    kc = K // 128
    return np.ascontiguousarray(
        (wT * scale).reshape(kc // 2, 2, 128, N).transpose(0, 2, 1, 3).reshape(kc // 2 * 128, 2 * N)
    ).astype(fp8)


def prepare_in_maps(x, in_proj_w, in_proj_b, out_w, out_b, w1, b1, w2, b2, ln_g, ln_b):
    import ml_dtypes
    bf16 = ml_dtypes.bfloat16
    fp8 = ml_dtypes.float8_e4m3
    pe = _pos_encoding_np(B, E)                      # (B, E)
    wq, wk, wv = in_proj_w[:E], in_proj_w[E:2 * E], in_proj_w[2 * E:]
    wqkvT = np.concatenate([wq.T, wk.T, wv.T], axis=1)   # (E, 3E)
    shared = {
        "wqkv8": _pack_pairs(wqkvT, fp8, WS),
        "woT": np.ascontiguousarray(out_w.T).astype(bf16),
        "w18": _pack_pairs(w1.T, fp8, WS),
        "w28": _pack_pairs(w2.T, fp8, WS),
        "bqkv2d": np.ascontiguousarray(in_proj_b.reshape(3 * EC, 128).T),
        # v bias folded through the out-projection: y = Wo(o0 + bv) + bo
        "bo2d": np.ascontiguousarray((out_b + out_w @ in_proj_b[2 * E:]).reshape(EC, 128).T),
        "b1_2d": np.ascontiguousarray(b1.reshape(HIDC, 128).T),
        "g2d": np.ascontiguousarray(ln_g.reshape(EC, 128).T),
        "bb2d": np.ascontiguousarray(ln_b.reshape(EC, 128).T),
        "bb2d_b2": np.ascontiguousarray((ln_b + b2).reshape(EC, 128).T),
        "ones_row": np.ones((1, 128), np.float32),
        "ones_col": np.full((128, 1), 1.0 / E, np.float32),  # LN stats: mean in one matmul
    }
    in_maps = []
    for c in range(NCORES):
        b, s = c // 2, c % 2
        xb = x[:, b, :] + pe[b][None, :]             # (L, E) with positional enc
        # rotate so own tokens are first: [own 1024 | other 1024]
        xrot = np.concatenate([xb[s * TOK:(s + 1) * TOK], xb[(1 - s) * TOK:(2 - s) * TOK]], axis=0)
        m = dict(shared)
        m["xa8"] = _pack_pairs(np.ascontiguousarray(xrot.T), fp8)       # fp8 pairs
        m["xw"] = np.ascontiguousarray(xb[s * TOK:(s + 1) * TOK].T)     # (E, TOK) f32
        in_maps.append(m)
    return in_maps


def assemble_output(results):
    y = np.empty((L, B, E), np.float32)
    for c in range(NCORES):
        b, s = c // 2, c % 2
        y[s * TOK:(s + 1) * TOK, b, :] = results[c]["yT"].T
    return y


def kernel(**inputs):
    inputs = {k: np.asarray(v, dtype=np.float32) for k, v in inputs.items()}
    nc = build_encoder()
    in_maps = prepare_in_maps(**inputs)
    res = run_bass_kernel_spmd(nc, in_maps, core_ids=list(range(NCORES)))
    return assemble_output(res.results)


# revision 12
# speedup vs baseline: 1.3815x; 1.1973x over previous
"""Trainium2 Bass kernel for nn_Encoder_88691074663154 (dense transformer encoder layer).

Strategy v4: batch x sequence sharding (core c = (b, s): batch c//2, sequence
half c%2), ZERO collectives; K/V recomputed per core for the full 2048 keys.
The host rotates each core's token order so its own 1024 query tokens are
always columns 0:1023 -> one uniform SPMD program.

Perf structure:
- QKV / AV / FFN1 / FFN2 matmuls run in fp8e4 with MatmulPerfMode.DoubleRow
  ([128, 2, N] pairs of contraction tiles, 0.5 cycles/col = 4x bf16).
  Weights are scaled by 64 on the host (fp8e4 subnormal floor); the 1/64 is
  folded into the psum readouts. Scores and out-proj stay bf16; LN f32r.
- The host pre-adds the positional encoding and ships x as fp8 pairs (xa8)
  plus an f32 residual copy (xw) - no on-chip x+pe ops at all.
- softmax exp splits across ACT (table exp, fp8 out) and DVE (Schraudolph:
  u8 = round(log2(e)*s + b) IS the fp8e4 bit pattern of exp(s/8); the ~3%
  error cancels through softmax normalization - verified end to end).
- Heads run in PAIRS with interleaved key loops: each scores-psum ring slot
  then has two heads of work between reuses, hiding the scores->exp->AV
  round-trip latency that otherwise serializes the attention loop.
- V bias is folded through the out-projection (bo += Wo @ bv) so the V
  readout is a pure scaled copy; b2 is folded into the LN1 bias of the bf16
  residual copy so the FFN2 readout stays one STT op.
"""
import os
import sys

sys.path.insert(0, "/opt/trn_rl_repo")

import numpy as np

import concourse.bacc as bacc
import concourse.mybir as mybir
import concourse.tile as tile
from concourse.bass_utils import run_bass_kernel_spmd

F32 = mybir.dt.float32
F32R = mybir.dt.float32r
BF16 = mybir.dt.bfloat16
FP8 = mybir.dt.float8e4
U8 = mybir.dt.uint8
AF = mybir.ActivationFunctionType
OP = mybir.AluOpType
PM = mybir.MatmulPerfMode

L, B, E, H, HD, HID = 2048, 4, 512, 8, 64, 2048
NCORES = 8
TOK = 1024                # own query tokens per core
EC = E // 128             # 4 feature chunks
ECP = EC // 2             # 2 feature chunk pairs
HIDC = HID // 128         # 16 hidden chunks
HIDP = HIDC // 2          # 8 hidden chunk pairs
KT = L // 128             # 16 key chunks
KP = KT // 2              # 8 key chunk pairs

WS = 64.0                 # fp8 weight scale
RWS = 1.0 / WS
# Schraudolph exp -> fp8e4 bits: u8 = round(EXP_A * s + EXP_B) where s is the
# raw qk psum (the /8 softmax scale is folded into EXP_A).
EXP_A = 12102203.161561485 / 2**20 / 8.0
EXP_B = 55.650580406188965 - 0.12

# exp engine schedule per head, indexed by half-chunk 2*t+g: A=ACT, D=DVE
EXPAT01 = "ADADADADADADADAD" * 2           # heads 0/1: A16 D16
EXPAT = "ADADADADADADADAD" + "ADADADADADADADAA"  # heads 2-7: A17 D15

_BUILD_CACHE = {}


def build_encoder():
    if "nc" in _BUILD_CACHE:
        return _BUILD_CACHE["nc"]
    nc = bacc.Bacc(None, num_devices=NCORES)

    # ---- DRAM parameters (per core) ----
    xa8_in = nc.declare_dram_parameter("xa8", [ECP * 128, 2 * L], FP8, isOutput=False)
    xw_in = nc.declare_dram_parameter("xw", [E, TOK], F32, isOutput=False)
    wqkv8_in = nc.declare_dram_parameter("wqkv8", [ECP * 128, 2 * 3 * E], FP8, isOutput=False)
    wo_in = nc.declare_dram_parameter("woT", [E, E], BF16, isOutput=False)
    w18_in = nc.declare_dram_parameter("w18", [ECP * 128, 2 * HID], FP8, isOutput=False)
    w28_in = nc.declare_dram_parameter("w28", [HIDP * 128, 2 * E], FP8, isOutput=False)
    bqkv_in = nc.declare_dram_parameter("bqkv2d", [128, 3 * EC], F32, isOutput=False)
    bo_in = nc.declare_dram_parameter("bo2d", [128, EC], F32, isOutput=False)
    b1_in = nc.declare_dram_parameter("b1_2d", [128, HIDC], F32, isOutput=False)
    g_in = nc.declare_dram_parameter("g2d", [128, EC], F32, isOutput=False)
    bb_in = nc.declare_dram_parameter("bb2d", [128, EC], F32, isOutput=False)
    bbb2_in = nc.declare_dram_parameter("bb2d_b2", [128, EC], F32, isOutput=False)
    ones_in = nc.declare_dram_parameter("ones_row", [1, 128], F32, isOutput=False)
    onesc_in = nc.declare_dram_parameter("ones_col", [128, 1], F32, isOutput=False)
    yT_out = nc.declare_dram_parameter("yT", [E, TOK], F32, isOutput=True)

    with tile.TileContext(nc) as tc:
        from contextlib import ExitStack
        with ExitStack() as ctx:
            pers = ctx.enter_context(tc.tile_pool(name="pers", bufs=1))

            # ---- persistent tiles ----
            onr = pers.tile([1, 128], F32R, tag="onr")
            onc = pers.tile([128, 1], F32R, tag="onc")
            bqkv = pers.tile([128, 3 * EC], F32, tag="bqkv")
            bo2d = pers.tile([128, EC], F32, tag="bo2d")
            b12d = pers.tile([128, HIDC], F32, tag="b12d")
            g2d = pers.tile([128, EC], F32, tag="g2d")
            bb2d = pers.tile([128, EC], F32, tag="bb2d")
            bbb2 = pers.tile([128, EC], F32, tag="bbb2")

            xw = [pers.tile([128, TOK], F32, tag=f"xw{k}", name=f"xw{k}") for k in range(EC)]
            kTt = [pers.tile([128, L], BF16, tag=f"kT{p}", name=f"kT{p}") for p in range(EC)]
            qTt = [pers.tile([128, TOK], BF16, tag=f"qT{p}", name=f"qT{p}") for p in range(EC)]
            # V pairs: [128, 2, H*66] fp8: per head 64 values + ones col (for the
            # softmax denominator) + 1 pad col (dual-fp8 Ldweights needs even /
            # 16B-aligned strides)
            vt8 = [pers.tile([128, 2 * H * 66], FP8, tag=f"vt{t}", name=f"vt{t}") for t in range(KP)]
            woT = [pers.tile([128, E], BF16, tag=f"woT{k}", name=f"woT{k}") for k in range(EC)]
            w18p = [pers.tile([128, 2 * HID], FP8, tag=f"w18{k}", name=f"w18{k}") for k in range(ECP)]
            w28p = [pers.tile([128, 2 * E], FP8, tag=f"w28{k}", name=f"w28{k}") for k in range(HIDP)]
            oT = [pers.tile([128, TOK], BF16, tag=f"oT{p}", name=f"oT{p}") for p in range(EC)]

            def pair(t):
                return t[:].rearrange("p (i x) -> p i x", i=2)

            # ========== Stage Q+A: QKV interleaved with attention ==========
            with tc.tile_pool(name="pq", bufs=1) as pq, \
                 tc.tile_pool(name="pa", bufs=1) as pa, \
                 tc.tile_pool(name="ps_sc", bufs=4, space="PSUM") as ps_sc, \
                 tc.tile_pool(name="ps_o", bufs=2, space="PSUM") as ps_o:
                wq8p = [pq.tile([128, 2 * 3 * E], FP8, tag=f"wq8{k}", name=f"wq8{k}") for k in range(ECP)]
                xap = [pq.tile([128, 2 * L], FP8, tag=f"xa{k}", name=f"xa{k}") for k in range(ECP)]

                for kp in range(ECP):
                    nc.sync.dma_start(xap[kp][:], xa8_in[kp * 128:(kp + 1) * 128, :])
                    nc.sync.dma_start(wq8p[kp][:], wqkv8_in[kp * 128:(kp + 1) * 128, :])
                nc.sync.dma_start(bqkv[:], bqkv_in[:])
                # ones columns of V (written once; disjoint from the value cols)
                for tp in range(KP):
                    vr = vt8[tp][:].rearrange("p (i h c) -> p i h c", i=2, c=66)
                    nc.vector.memset(vr[:, :, :, 64:65], 1.0)

                def kq_group(p, g, is_k):
                    # one 512-column group of K (g<4) or Q (g<2) for head pair p
                    sl = slice(g * 512, (g + 1) * 512)
                    col = E + p * 128 if is_k else p * 128
                    dst = kTt[p] if is_k else qTt[p]
                    bcol = EC + p if is_k else p
                    ps = ps_sc.tile([128, 512], F32, tag="sc", name="ps_kq")
                    for kp in range(ECP):
                        w = pair(wq8p[kp])[:, :, col:col + 128]
                        nc.tensor.matmul(ps[:], w, pair(xap[kp])[:, :, sl],
                                         start=(kp == 0), stop=(kp == ECP - 1),
                                         perf_mode=PM.DoubleRow)
                    if is_k:
                        # ACT: identity(ps/WS + b)
                        nc.scalar.activation(dst[:, sl], ps[:], AF.Identity,
                                             bias=bqkv[:, bcol:bcol + 1], scale=RWS)
                    else:
                        nc.vector.tensor_scalar(dst[:, sl], ps[:], RWS,
                                                bqkv[:, bcol:bcol + 1],
                                                op0=OP.mult, op1=OP.add)

                def kq_chunk(p):
                    for g in range(4):
                        kq_group(p, g, True)
                    for g in range(2):
                        kq_group(p, g, False)

                def v_chunk(t):
                    ps = ps_sc.tile([128, 512], F32, tag="sc", name="ps_v")
                    for kp in range(ECP):
                        nc.tensor.matmul(
                            ps[:], pair(xap[kp])[:, :, t * 128:(t + 1) * 128],
                            pair(wq8p[kp])[:, :, 2 * E:3 * E],
                            start=(kp == 0), stop=(kp == ECP - 1),
                            perf_mode=PM.DoubleRow)
                    # V bias folded into bo2d on the host -> pure scaled copy
                    vr = vt8[t // 2][:].rearrange("p (i h c) -> p i h c", i=2, c=66)
                    dst = vr[:, t % 2, :, 0:64]
                    src = ps[:].rearrange("t (h c) -> t h c", c=64)
                    if t % 2 == 0:
                        with nc.allow_low_precision(reason="fp8 V for DR matmul"):
                            nc.vector.tensor_scalar_mul(dst, src, RWS)
                    else:
                        nc.scalar.activation(dst, src, AF.Copy, scale=RWS)

                def normalize(h, pso):
                    # 1/rowsum (psum row 64); replicate across partitions on gpsimd
                    hp, ro = h // 2, (h % 2) * 64
                    rr = pa.tile([1, TOK], F32R, tag="rr", bufs=2, name="rr")
                    with nc.allow_low_precision(reason="softmax denom rounded to f32r"):
                        nc.vector.reciprocal(rr[:], pso[64:65, :])
                    rsb = pa.tile([64, TOK], F32R, tag="rsb", bufs=2, name="rsb")
                    nc.gpsimd.partition_broadcast(rsb[:], rr[:])
                    nc.vector.tensor_mul(oT[hp][ro:ro + 64, :], pso[0:64, :],
                                         rsb[:].bitcast(F32))

                for p in range(EC):
                    kq_chunk(p)

                def s_exp(h, t, ex, v_inline=False):
                    # scores for key chunk t + exp into ex[:, t%2, :] (fp8),
                    # emitted as two independent 512-query halves so the psum
                    # slots are 1 bank each and the ring can be 4 deep
                    hp, ro = h // 2, (h % 2) * 64
                    if v_inline:
                        v_chunk(t)
                    for g in range(2):
                        sl = slice(g * 512, (g + 1) * 512)
                        pssc = ps_sc.tile([128, 512], F32, tag="sc", name="ps_sc")
                        nc.tensor.matmul(
                            pssc[:],
                            kTt[hp][ro:ro + 64, t * 128:(t + 1) * 128],
                            qTt[hp][ro:ro + 64, sl],
                            start=True, stop=True)
                        dst = pair(ex)[:, t % 2, sl]
                        kind = (EXPAT01 if h < 2 else EXPAT)[2 * t + g]
                        if kind == "A":
                            nc.scalar.activation(dst, pssc[:], AF.Exp, scale=0.125)
                        else:
                            with nc.allow_low_precision(reason="schraudolph exp to fp8"):
                                nc.vector.tensor_scalar(dst.bitcast(U8), pssc[:],
                                                        EXP_A, EXP_B,
                                                        op0=OP.mult, op1=OP.add)

                def av_pair(h, tp, pso, ex):
                    vv = pair(vt8[tp])[:, :, h * 66:h * 66 + 65]
                    exr = pair(ex)
                    for g in range(2):
                        sl = slice(g * 512, (g + 1) * 512)
                        nc.tensor.matmul(pso[:, sl], vv, exr[:, :, sl],
                                         start=(tp == 0), stop=(tp == KP - 1),
                                         perf_mode=PM.DoubleRow)

                def new_ex():
                    return pa.tile([128, 2 * TOK], FP8, tag="ex", bufs=4, name="ex")

                def head_pair(h0, h1, v_inline=False):
                    # two heads share one interleaved key loop: the scores-psum
                    # ring slots get 2 heads of work between reuses, hiding the
                    # scores->exp->AV round trip
                    pso0 = ps_o.tile([65, TOK], F32, tag="pso", name="ps_av")
                    pso1 = ps_o.tile([65, TOK], F32, tag="pso", name="ps_av")
                    ex0 = new_ex()
                    s_exp(h0, 0, ex0, v_inline=v_inline)
                    s_exp(h0, 1, ex0, v_inline=v_inline)
                    ex1 = new_ex()
                    s_exp(h1, 0, ex1)
                    s_exp(h1, 1, ex1)
                    for tp in range(KP):
                        if tp + 1 < KP:
                            nx0 = new_ex()
                            s_exp(h0, 2 * tp + 2, nx0, v_inline=v_inline)
                            s_exp(h0, 2 * tp + 3, nx0, v_inline=v_inline)
                            nx1 = new_ex()
                            s_exp(h1, 2 * tp + 2, nx1)
                            s_exp(h1, 2 * tp + 3, nx1)
                        else:
                            nx0 = nx1 = None
                        av_pair(h0, tp, pso0, ex0)
                        if tp == KP - 1:
                            normalize(h0, pso0)
                        av_pair(h1, tp, pso1, ex1)
                        if tp == KP - 1:
                            normalize(h1, pso1)
                        ex0, ex1 = nx0, nx1

                head_pair(0, 1, v_inline=True)
                # post-phase weights + residual (DMA is idle during attention)
                for k in range(EC):
                    nc.sync.dma_start(woT[k][:], wo_in[k * 128:(k + 1) * 128, :])
                    nc.sync.dma_start(xw[k][:], xw_in[k * 128:(k + 1) * 128, :])
                for kp in range(ECP):
                    nc.sync.dma_start(w18p[kp][:], w18_in[kp * 128:(kp + 1) * 128, :])
                for kp in range(HIDP):
                    nc.sync.dma_start(w28p[kp][:], w28_in[kp * 128:(kp + 1) * 128, :])
                nc.sync.dma_start(bo2d[:], bo_in[:])
                nc.sync.dma_start(b12d[:], b1_in[:])
                nc.sync.dma_start(g2d[:], g_in[:])
                nc.sync.dma_start(bb2d[:], bb_in[:])
                nc.sync.dma_start(bbb2[:], bbb2_in[:])
                nc.sync.dma_start(onr[:], ones_in[:].bitcast(F32R))
                nc.sync.dma_start(onc[:], onesc_in[:].bitcast(F32R))

                head_pair(2, 3)
                head_pair(4, 5)
                head_pair(6, 7)

            # ================= Stage P: out-proj + LN1 + FFN + LN2 =================
            with tc.tile_pool(name="pp", bufs=1) as pp, \
                 tc.tile_pool(name="ps_mm", bufs=2, space="PSUM") as ps_mm, \
                 tc.tile_pool(name="ps_ln", bufs=1, space="PSUM") as ps_ln:

                xres = [pp.tile([128, TOK], F32R, tag=f"xres{m}", name=f"xres{m}") for m in range(EC)]
                x1bb = [pp.tile([128, TOK], BF16, tag=f"x1b{m}", name=f"x1b{m}") for m in range(EC)]
                x18 = [pp.tile([128, 2 * TOK], FP8, tag=f"x18{m}", name=f"x18{m}") for m in range(ECP)]
                yt = [pp.tile([128, TOK], F32, tag=f"yt{m}", name=f"yt{m}") for m in range(EC)]
                hT8 = [pp.tile([128, 2 * TOK], FP8, tag=f"hT{m}", name=f"hT{m}") for m in range(HIDP)]

                _ln = {}

                def ln_stats(src, gh):
                    """Stats + per-token scalar chain for one column half."""
                    sl = slice(gh * 512, (gh + 1) * 512)
                    sqs = []
                    for k in range(EC):
                        sq = pp.tile([128, 512], F32R, tag="sq", bufs=2, name="sq")
                        eng = nc.vector if k % 2 == 0 else nc.gpsimd
                        with nc.allow_low_precision(reason="LN variance in f32r"):
                            eng.tensor_mul(sq[:], src[k][:, sl], src[k][:, sl])
                        sqs.append(sq)
                    pss = ps_ln.tile([1, 512], F32, tag="pss", name="ps_mean")
                    for k in range(EC):
                        nc.tensor.matmul(pss[:], onc[:], src[k][:, sl],
                                         start=(k == 0), stop=(k == EC - 1))
                    pss2 = ps_ln.tile([1, 512], F32, tag="pss2", name="ps_var")
                    for k in range(EC):
                        nc.tensor.matmul(pss2[:], onc[:], sqs[k][:],
                                         start=(k == 0), stop=(k == EC - 1))
                    # ones_col carries 1/E, so pss/pss2 are already E[x], E[x^2]
                    rows = pp.tile([1, 2 * 512], F32, tag="lnrows", bufs=2, name="lnrows")
                    rowsr = pp.tile([1, 2 * 512], F32R, tag="lnrowsr", bufs=2, name="lnrowsr")
                    mu2 = rows[0:1, 0:512]
                    rec = rows[0:1, 512:1024]
                    mur = rowsr[0:1, 0:512]
                    rsq = rowsr[0:1, 512:1024]
                    nc.vector.tensor_copy(mur, pss[:])
                    nc.scalar.activation(mu2, pss[:], AF.Square)
                    nc.vector.scalar_tensor_tensor(rec, pss2[:], 1.0, mu2,
                                                   op0=OP.mult, op1=OP.subtract)
                    nc.vector.tensor_scalar_add(rec, rec, 1e-5)
                    nc.vector.reciprocal(rec, rec)
                    nc.scalar.activation(rsq, rec, AF.Sqrt)
                    _ln[gh] = (mur, rsq)

                # per-chunk engine assignment for the LN normalize chains
                CHAIN = [None, None, None, None]
                DSTE = ["A", "D", "A", "D"]

                def ln_finish(src, dst, gh, dma=False, fp8_dst=None, fold_b2=False):
                    """Replicate + normalize one column half; chains spread over
                    gpsimd/DVE, affine writes over ACT/DVE."""
                    sl = slice(gh * 512, (gh + 1) * 512)
                    mur, rsq = _ln[gh]
                    psm = ps_ln.tile([128, TOK], F32, tag="psm", name="ps_lnrep")
                    nc.tensor.matmul(psm[:, 0:512], onr[:], mur, start=True, stop=True)
                    nc.tensor.matmul(psm[:, 512:1024], onr[:], rsq, start=True, stop=True)
                    msb = pp.tile([128, 512], F32, tag="lnmsb", bufs=1, name="lnmsb")
                    rsb = pp.tile([128, 512], F32, tag="lnrsb", bufs=1, name="lnrsb")
                    nc.vector.tensor_copy(msb[:], psm[:, 0:512])
                    nc.vector.tensor_copy(rsb[:], psm[:, 512:1024])
                    bias = bbb2 if fold_b2 else bb2d
                    for k in range(EC):
                        eng = nc.vector if k == 1 else nc.gpsimd
                        t1 = pp.tile([128, 512], F32, tag="t1", bufs=2, name="t1")
                        eng.tensor_sub(t1[:], src[k][:, sl].bitcast(F32), msb[:])
                        t2 = pp.tile([128, 512], F32, tag="t2", bufs=2, name="t2")
                        eng.tensor_mul(t2[:], t1[:], rsb[:])
                        if DSTE[k] == "A":
                            nc.scalar.activation(dst[k][:, sl], t2[:], AF.Identity,
                                                 bias=bias[:, k:k + 1], scale=g2d[:, k:k + 1])
                        else:
                            nc.vector.tensor_scalar(dst[k][:, sl], t2[:], g2d[:, k:k + 1],
                                                    bias[:, k:k + 1], op0=OP.mult, op1=OP.add)
                        if fp8_dst is not None:
                            with nc.allow_low_precision(reason="fp8 copy for DR matmul"):
                                nc.gpsimd.tensor_scalar(
                                    pair(fp8_dst[k // 2])[:, k % 2, sl], t2[:],
                                    g2d[:, k:k + 1], bb2d[:, k:k + 1],
                                    op0=OP.mult, op1=OP.add)
                        if dma:
                            nc.sync.dma_start(yT_out[k * 128:(k + 1) * 128, sl], dst[k][:, sl])

                def proj(g):
                    sl = slice(g * 512, (g + 1) * 512)
                    for m in range(EC):
                        pst = ps_mm.tile([128, 512], F32, tag="mm", name="ps_proj")
                        for k in range(EC):
                            nc.tensor.matmul(pst[:], woT[k][:, m * 128:(m + 1) * 128],
                                             oT[k][:, sl], start=(k == 0), stop=(k == EC - 1))
                        nc.vector.scalar_tensor_tensor(
                            xres[m][:, sl], pst[:], bo2d[:, m:m + 1], xw[m][:, sl],
                            op0=OP.add, op1=OP.add)

                def ffn1(g):
                    sl = slice(g * 512, (g + 1) * 512)
                    for m in range(HIDC):
                        psf = ps_mm.tile([128, 512], F32, tag="mm", name="ps_f1")
                        for kp in range(ECP):
                            nc.tensor.matmul(psf[:], pair(w18p[kp])[:, :, m * 128:(m + 1) * 128],
                                             pair(x18[kp])[:, :, sl],
                                             start=(kp == 0), stop=(kp == ECP - 1),
                                             perf_mode=PM.DoubleRow)
                        nc.scalar.activation(pair(hT8[m // 2])[:, m % 2, sl], psf[:],
                                             AF.Gelu, bias=b12d[:, m:m + 1], scale=RWS)

                def ffn2(g):
                    # b2 is pre-folded into x1bb's bias; readout is one STT
                    sl = slice(g * 512, (g + 1) * 512)
                    for m in range(EC):
                        psg = ps_mm.tile([128, 512], F32, tag="mm", name="ps_f2")
                        for kp in range(HIDP):
                            nc.tensor.matmul(psg[:], pair(w28p[kp])[:, :, m * 128:(m + 1) * 128],
                                             pair(hT8[kp])[:, :, sl],
                                             start=(kp == 0), stop=(kp == HIDP - 1),
                                             perf_mode=PM.DoubleRow)
                        nc.vector.scalar_tensor_tensor(
                            xres[m][:, sl], psg[:], RWS, x1bb[m][:, sl],
                            op0=OP.mult, op1=OP.add)

                proj(0)
                ln_stats(xres, 0)
                proj(1)
                ln_finish(xres, x1bb, 0, fp8_dst=x18, fold_b2=True)
                ln_stats(xres, 1)
                ffn1(0)
                ln_finish(xres, x1bb, 1, fp8_dst=x18, fold_b2=True)
                ffn1(1)
                ffn2(0)
                ln_stats(xres, 0)
                ffn2(1)
                ln_finish(xres, yt, 0, dma=True)
                ln_stats(xres, 1)
                ln_finish(xres, yt, 1, dma=True)

    nc.compile()
    _BUILD_CACHE["nc"] = nc
    return nc


def _pos_encoding_np(S, Emb):
    t = np.arange(S, dtype=np.float32)[:, None]
    i = np.arange(Emb, dtype=np.float32)[None, :]
    even = np.sin((t + 1.0) * np.power(np.float32(10000.0), -i / Emb))
    odd = np.cos((t + 1.0) * np.power(np.float32(10000.0), -(i + 1.0) / Emb))
    return np.where(np.arange(Emb)[None, :] % 2 == 0, even, odd).astype(np.float32)


def _pack_pairs(wT, fp8, scale=1.0):
    """(Kc*128, N) -> (Kc/2*128, 2*N): row (kp*128+p), col (i*N+c) = wT[(2kp+i)*128+p, c]."""
    K, N = wT.shape
    kc = K // 128
    return np.ascontiguousarray(
        (wT * scale).reshape(kc // 2, 2, 128, N).transpose(0, 2, 1, 3).reshape(kc // 2 * 128, 2 * N)
    ).astype(fp8)


def prepare_in_maps(x, in_proj_w, in_proj_b, out_w, out_b, w1, b1, w2, b2, ln_g, ln_b):
    import ml_dtypes
    bf16 = ml_dtypes.bfloat16
    fp8 = ml_dtypes.float8_e4m3
    pe = _pos_encoding_np(B, E)                      # (B, E)
    wq, wk, wv = in_proj_w[:E], in_proj_w[E:2 * E], in_proj_w[2 * E:]
    wqkvT = np.concatenate([wq.T, wk.T, wv.T], axis=1)   # (E, 3E)
    shared = {
        "wqkv8": _pack_pairs(wqkvT, fp8, WS),
        "woT": np.ascontiguousarray(out_w.T).astype(bf16),
        "w18": _pack_pairs(w1.T, fp8, WS),
        "w28": _pack_pairs(w2.T, fp8, WS),
        "bqkv2d": np.ascontiguousarray(in_proj_b.reshape(3 * EC, 128).T),
        # v bias folded through the out-projection: y = Wo(o0 + bv) + bo
        "bo2d": np.ascontiguousarray((out_b + out_w @ in_proj_b[2 * E:]).reshape(EC, 128).T),
        "b1_2d": np.ascontiguousarray(b1.reshape(HIDC, 128).T),
        "g2d": np.ascontiguousarray(ln_g.reshape(EC, 128).T),
        "bb2d": np.ascontiguousarray(ln_b.reshape(EC, 128).T),
        "bb2d_b2": np.ascontiguousarray((ln_b + b2).reshape(EC, 128).T),
        "ones_row": np.ones((1, 128), np.float32),
        "ones_col": np.full((128, 1), 1.0 / E, np.float32),  # LN stats: mean in one matmul
    }
    in_maps = []
    for c in range(NCORES):
        b, s = c // 2, c % 2
        xb = x[:, b, :] + pe[b][None, :]             # (L, E) with positional enc
        # rotate so own tokens are first: [own 1024 | other 1024]
        xrot = np.concatenate([xb[s * TOK:(s + 1) * TOK], xb[(1 - s) * TOK:(2 - s) * TOK]], axis=0)
        m = dict(shared)
        m["xa8"] = _pack_pairs(np.ascontiguousarray(xrot.T), fp8)       # fp8 pairs
        m["xw"] = np.ascontiguousarray(xb[s * TOK:(s + 1) * TOK].T)     # (E, TOK) f32
        in_maps.append(m)
    return in_maps


def assemble_output(results):
    y = np.empty((L, B, E), np.float32)
    for c in range(NCORES):
        b, s = c // 2, c % 2
        y[s * TOK:(s + 1) * TOK, b, :] = results[c]["yT"].T
    return y


def kernel(**inputs):
    inputs = {k: np.asarray(v, dtype=np.float32) for k, v in inputs.items()}
    nc = build_encoder()
    in_maps = prepare_in_maps(**inputs)
    res = run_bass_kernel_spmd(nc, in_maps, core_ids=list(range(NCORES)))
    return assemble_output(res.results)
